# revision 39
# baseline (speedup 1.0000x reference)
"""Causal multi-head self-attention with RoPE on 8 TRN2 NeuronCores.

Sharding: core c handles batch b = c // 4 and heads [4*(c%4), 4*(c%4)+4).
All cores run one SPMD Bass program; per-core behavior comes entirely from
the data (pre-sliced weights, per-batch activations). Each core computes its
4 heads' attention and the partial output projection y^T = W_o_slice^T @ out;
the host sums the 4 partials per batch (the "all-reduce" of the TP split).

Device layout is feature-major throughout: x^T [D, S] feeds QKV as the
moving operand; scores are computed transposed (k on partitions, q free) so
the softmax denominator falls out of a ones-row appended to V in the PV
matmul, and the attention output emerges as out^T [d, q], which is exactly
the moving operand the output projection needs. RoPE is applied on the QKV
PSUM with a pair-swap stream shuffle + host-precomputed cos/sin tables.

The emission is software-pipelined: attention slots for query tile t (whose
per-slot rate is bounded by the Activation engine's exp) are interleaved
with the QKV/V projection chains of tile t+1 and the output projection of
tile t-1, so the Tensor engine always has filler matmuls while exp catches
up. Compute dtype bf16 (fp32 accumulate), f32 in, bf16 partials out (host
accumulates the 4 per-batch partials in f32).

The QKV projection runs in fp8(e4m3) DoubleRow perf mode (0.5 PE cycles per
output row, two 128-deep contraction subtiles per instruction) using a
3-term residual expansion x·W ~= x8·w8 + r8·w8 + x8·s8 with r8 = fp8(x-x8),
s8 = fp8(64W - w8); W is pre-scaled by 64 so its residual clears the e4m3
subnormal floor. All quantization happens host-side; the 1/64 descale is
folded into the cos/sin tables for Q/K and into the V psum-copy multiplier.
"""

import sys

sys.path.insert(0, "/opt/trn_rl_repo")

import numpy as np
import ml_dtypes

import concourse.bass as bass
import concourse.bacc as bacc
import concourse.mybir as mybir
import concourse.tile as tile
from concourse.bass_utils import run_bass_kernel_spmd

B, S, D = 2, 2048, 1024
H, DK = 16, 64
THETA = 10000.0
HPC = 4  # heads per core
P = 128
KO = D // P  # 8 contraction subtiles for the projections
QTILE = 512
NQ = S // QTILE  # 4 query tiles
NKT = S // P  # 16 key-token tiles
N_CORES = 8
BF = ml_dtypes.bfloat16

_PAIRSWAP = [i + 1 if i % 2 == 0 else i - 1 for i in range(32)]

F8 = ml_dtypes.float8_e4m3
WSCALE = 64.0  # host pre-scale on W_qkv so fp8 residuals stay out of subnormals

_CACHE = {}


def _build_nc(reps=1, loop=False, probe=(), opts=None):
    probe = set(probe)
    opts = dict(opts or {})
    vcopy_eng = opts.get("vcopy", "act")
    ycopy_eng = opts.get("ycopy", "dve")
    pvdepth = opts.get("pvdepth", 6)
    probs_bufs = opts.get("probs_bufs", 10)
    rope_bufs = opts.get("rope_bufs", 8)
    yp_bufs = opts.get("yp_bufs", 8)
    on_bufs = opts.get("on_bufs", 3)
    diag_first = opts.get("diag_first", False)
    n_warm = opts.get("n_warm", 12)
    f32 = mybir.dt.float32
    bf16 = mybir.dt.bfloat16
    fp8 = mybir.dt.float8e4
    Exp = mybir.ActivationFunctionType.Exp
    DR = mybir.MatmulPerfMode.DoubleRow

    nc = bacc.Bacc()
    # paired-value/residual tensors are combined on dim1 so each slice is a
    # single DMA (HWDGE generation is a serial 625ns/DMA device); x is blocked
    # by 256-token chunks so every DMA slice is a 2KB contiguous run (descs
    # under 512B pay a 2x DMA latency multiplier)
    NB = S // 256
    xT_d = nc.dram_tensor("xT2", [P, 2, NB, KO, 256], fp8, kind="ExternalInput")
    wqk_d = nc.dram_tensor("wqk2", [P, 2, 4, KO, P], fp8, kind="ExternalInput")
    wv_d = nc.dram_tensor("wv2", [P, 2, KO, HPC * DK], fp8, kind="ExternalInput")
    wo_d = nc.dram_tensor("wo", [P, 2, D], bf16, kind="ExternalInput")
    wo2_d = nc.dram_tensor("wo2", [P, 2, 2, D], fp8, kind="ExternalInput")
    cs_d = nc.dram_tensor("cossin", [P, 2, S], bf16, kind="ExternalInput")
    mask_d = nc.dram_tensor("masks", [P, P], bf16, kind="ExternalInput")
    out_d = nc.dram_tensor("out_t", [P, KO, S], bf16, kind="ExternalOutput")

    with tile.TileContext(nc) as tc:
        with (
            tc.tile_pool(name="const", bufs=1) as cp,
            tc.tile_pool(name="rope", bufs=rope_bufs) as ropep,
            tc.tile_pool(name="probs", bufs=probs_bufs) as probsp,
            tc.tile_pool(name="onp", bufs=on_bufs) as onormp,
            tc.tile_pool(name="on8", bufs=on_bufs) as on8p,
            tc.tile_pool(name="yp", bufs=yp_bufs) as yp,
            tc.tile_pool(name="ysb", bufs=2) as ysbp,
            tc.tile_pool(name="ps_s", bufs=2, space="PSUM") as ps_s,
            tc.tile_pool(name="ps_o", bufs=2, space="PSUM") as ps_o,
            tc.tile_pool(name="ps_m", bufs=2, space="PSUM") as ps_m,
        ):
            xt2 = cp.tile([P, 2, NB, KO, 256], fp8, tag="xT2")
            wqk2 = cp.tile([P, 2, 4, KO, P], fp8, tag="wqk2")
            wv2 = cp.tile([P, 2, KO, HPC * DK], fp8, tag="wv2")
            wo = cp.tile([P, 2, D], bf16, tag="wo")
            wo2 = cp.tile([P, 2, 2, D], fp8, tag="wo2")
            cossin = cp.tile([P, 2, S], bf16, tag="cossin")
            maskt = cp.tile([P, P], bf16, tag="mask")
            vones = cp.tile([P, NKT, HPC, DK + 1], bf16, tag="vones")
            qsb = cp.tile([P, 2, S], bf16, tag="qsb")
            ksb = cp.tile([P, 2, S], bf16, tag="ksb")
            scratch = cp.tile([P, 256], bf16, tag="scr")

            # PE pre-ramp: the p-state model runs matmuls at reduced clock
            # until the engine has been continuously busy ~3us. Warm it up on
            # scratch data while the first input DMAs are in flight; the ramp
            # then carries into the real matmul stream with no idle gap.
            # the ones row is 1/8 so onorm comes out as 8*att, matching the
            # x8 pre-scale of the fp8 output-projection weights
            nc.gpsimd.memset(scratch[:], 0.0)
            nc.vector.memset(vones[:, :, :, DK:DK + 1], 0.125)
            if n_warm:
                wps = ps_s.tile([P, 256], f32, tag="s", name="warm")
                for i in range(n_warm):
                    nc.tensor.matmul(
                        wps[:],
                        scratch[:, 0:P],
                        scratch[:],
                        start=(i == 0),
                        stop=(i == n_warm - 1),
                    )

            # input DMAs, all on the SP queue. Each DMA costs one 625ns HWDGE
            # slot and its transfer serializes on the single DMA device, so
            # order = need-time: the first QKV chain's operands in fine grain
            # (the chain starts as soon as ko 0 lands and is paced by the
            # rest), then everything else batched coarse.
            nc.sync.dma_start(wqk2[:, :, 0], wqk_d[:, :, 0])
            nc.sync.dma_start(xt2[:, :, 0], xT_d[:, :, 0])
            nc.sync.dma_start(wqk2[:, :, 1:4], wqk_d[:, :, 1:4])
            nc.sync.dma_start(cossin[:, :, 0:QTILE], cs_d[:, :, 0:QTILE])
            nc.sync.dma_start(xt2[:, :, 1], xT_d[:, :, 1])
            nc.sync.dma_start(maskt[:], mask_d[:])
            nc.sync.dma_start(wv2[:], wv_d[:])
            for nt in range(1, NQ):
                sl = slice(nt * QTILE, (nt + 1) * QTILE)
                nc.sync.dma_start(xt2[:, :, 2 * nt:2 * nt + 2], xT_d[:, :, 2 * nt:2 * nt + 2])
                nc.sync.dma_start(cossin[:, :, sl], cs_d[:, :, sl])
                if nt == 1:
                    nc.sync.dma_start(wo2[:], wo2_d[:])
                if nt == 2:
                    nc.sync.dma_start(wo[:], wo_d[:])

            def qkv_chain(nt, mt, lo=0, hi=QTILE, cpy_act=False):
                """One 128-col QKV chain + psum copy + rope to qsb/ksb.

                3-term fp8 DoubleRow: x8·w8 + r8·w8 + x8·s8 in 256-col
                chunks (DoubleRow moving free = 2N <= 512)."""
                nsl = slice(nt * QTILE + lo, nt * QTILE + hi)
                w = hi - lo
                pqk = ps_m.tile([P, w], f32, tag="m", name="pqk")
                for c0 in range(0, w, 256):
                    cw = min(256, w - c0)
                    blk = (nt * QTILE + lo + c0) // 256
                    ni = 0
                    for jw, jx in ((0, 0), (0, 1), (1, 0)):
                        for t in range(KO // 2):
                            nc.tensor.matmul(
                                pqk[:, c0:c0 + cw],
                                wqk2[:, jw, mt, 2 * t:2 * t + 2, :],
                                xt2[:, jx, blk, 2 * t:2 * t + 2, 0:cw],
                                start=(ni == 0),
                                stop=(ni == 3 * (KO // 2) - 1),
                                perf_mode=DR,
                            )
                            ni += 1
                cpy = ropep.tile([P, w], bf16, tag="cpy", name="cpy")
                if cpy_act:  # prologue: the exp stream hasn't started yet
                    nc.scalar.copy(cpy[:], pqk[:])
                else:
                    nc.vector.tensor_copy(cpy[:], pqk[:])
                t0 = ropep.tile([P, w], bf16, tag="t0", name="t0")
                sw = ropep.tile([P, w], bf16, tag="sw", name="sw")
                u0 = ropep.tile([P, w], bf16, tag="u0", name="u0")
                nc.vector.tensor_mul(t0[:], cpy[:], cossin[:, 0, nsl])
                nc.vector.stream_shuffle(sw[:], cpy[:], _PAIRSWAP)
                nc.vector.tensor_mul(u0[:], sw[:], cossin[:, 1, nsl])
                dst = qsb if mt < 2 else ksb
                nc.vector.tensor_add(dst[:, mt % 2, nsl], t0[:], u0[:])

            def v_chain(nt, tt, cpy_act=False):
                """V projection for one 128-token subtile (3-term fp8 DR).
                The psum holds 64·v; the copy descales by 1/64."""
                kt = nt * 4 + tt
                blk, off = kt // 2, (kt % 2) * P
                pv = ps_m.tile([P, HPC, DK], f32, tag="m", name="pv")
                ni = 0
                for jx, jw in ((0, 0), (1, 0), (0, 1)):
                    for t in range(KO // 2):
                        nc.tensor.matmul(
                            pv[:],
                            xt2[:, jx, blk, 2 * t:2 * t + 2, off:off + P],
                            wv2[:, jw, 2 * t:2 * t + 2, :],
                            start=(ni == 0),
                            stop=(ni == 3 * (KO // 2) - 1),
                            perf_mode=DR,
                        )
                        ni += 1
                if vcopy_eng == "dve" and not cpy_act:
                    nc.vector.tensor_scalar_mul(
                        vones[:, kt, :, 0:DK], pv[:], 1.0 / WSCALE
                    )
                else:
                    nc.scalar.mul(vones[:, kt, :, 0:DK], pv[:], 1.0 / WSCALE)

            def phase_b_fillers(nt):
                return [lambda mt=mt: qkv_chain(nt, mt) for mt in range(4)] + [
                    lambda tt=tt: v_chain(nt, tt) for tt in range(4)
                ]

            def oproj_fillers(qt, onorm_pack, pools=((ps_m, "m"),)):
                """Output projection chains for query tile qt (3-term fp8
                DoubleRow; psum lands at 64x so copies descale by 1/64).
                The later tiles run near the kernel tail where the exp
                stream has ended, so their copies alternate onto the idle
                Activation engine and their output DMAs are split to keep
                the last transfer small."""
                onorm, onorm8, rho8 = onorm_pack
                qsl = slice(qt * QTILE, (qt + 1) * QTILE)
                late = qt >= NQ - 2
                last = qt == NQ - 1
                ysb = ysbp.tile([P, KO, QTILE], bf16, tag="y", name="ysb")

                def ochain(ot):
                    pool, ptag = pools[ot % len(pools)]
                    py = pool.tile([P, QTILE], f32, tag=ptag, name="py")
                    otsl = slice(ot * P, (ot + 1) * P)
                    for c0 in (0, 256):
                        for ni, (j, mv) in enumerate(
                            ((0, onorm8), (0, rho8), (1, onorm8))
                        ):
                            nc.tensor.matmul(
                                py[:, c0:c0 + 256],
                                wo2[:, j, 0:2, otsl],
                                mv[:, 0:2, c0:c0 + 256],
                                start=(ni == 0),
                                stop=(ni == 2),
                                perf_mode=DR,
                            )
                    if "noy" in probe:
                        return
                    # Activation copies only where the exp stream has ended
                    # (the held-back tail chains); mid-round copies stay on
                    # DVE so they don't stretch the exp-bound final round
                    use_act = ycopy_eng == "act" or (qt == NQ - 2 and ot >= 2)
                    if use_act:
                        nc.scalar.mul(ysb[:, ot, :], py[:], 1.0 / WSCALE)
                    else:
                        nc.vector.tensor_scalar_mul(
                            ysb[:, ot, :], py[:], 1.0 / WSCALE
                        )
                    # split the late tiles' output DMAs so the tail after the
                    # last matmul is a small transfer, not a whole tile
                    if late and ot == 3:
                        nc.sync.dma_start(out_d[:, 0:4, qsl], ysb[:, 0:4, :])
                    elif last and ot == 5:
                        nc.sync.dma_start(out_d[:, 4:6, qsl], ysb[:, 4:6, :])
                    elif last and ot == 6:
                        nc.sync.dma_start(out_d[:, 6:7, qsl], ysb[:, 6:7, :])
                    elif last and ot == KO - 1:
                        nc.sync.dma_start(out_d[:, 7:8, qsl], ysb[:, 7:8, :])
                    elif not last and ot == KO - 1:
                        if late:
                            nc.sync.dma_start(out_d[:, 4:8, qsl], ysb[:, 4:8, :])
                        else:
                            nc.sync.dma_start(out_d[:, :, qsl], ysb[:])

                return [lambda ot=ot: ochain(ot) for ot in range(KO)]

            def run_round(qt, fillers, pre=(), defer_at=4, keep_tail=2,
                          final=False):
                """Attention for query tile qt, interleaved with fillers.

                ``pre`` holds deferred closures (the previous round's softmax
                normalization chains): they are emitted a few slots in, so
                they don't head-of-line-block the DVE queue ahead of this
                round's first diagonal mask multiplies. This round's own norm
                chains are returned as closures for the next round (the last
                round emits them inline). Filler pacing is weighted by each
                slot's PE deficit: a diagonal slot has little matmul work but
                a full exp, so it gets more filler coverage.
                """
                onorm = onormp.tile([P, 2, QTILE], bf16, tag="on", name="onorm")
                if qt < NQ - 1:
                    # fp8 quantized onorm + residual for the fp8 output
                    # projection (produced on the otherwise-idle Pool engine)
                    onorm8 = on8p.tile([P, 2, QTILE], fp8, tag="o8", name="on8")
                    rho8 = on8p.tile([P, 2, QTILE], fp8, tag="r8", name="rho8")
                else:
                    onorm8 = rho8 = None
                nkt = 4 * (qt + 1)
                nfill = len(fillers)
                emitted = [0]
                # deferred closures keyed by the slot index that releases them
                deferq = {defer_at: list(pre)}

                # per-slot PE deficit weight: full-slot ACT time is roughly
                # constant, PE slot work scales with the causal width
                weights = []
                for hp in range(2):
                    for kt in (range(4 * qt, nkt) if diag_first else range(nkt)):
                        r = kt - 4 * qt
                        rq = max(r, 0) * P
                        weights.append(1.0 + 3.0 * rq / QTILE)
                    if diag_first:
                        weights.extend([1.0] * (4 * qt))
                total_w = sum(weights)
                nslots = len(weights)
                cum = 0.0

                front = 1.5 if final else 1.0
                # fillers may consume onorm written by the deferred norm
                # closures in ``pre`` — hold them until those are emitted
                fill_gate = defer_at if pre else 0

                def pace(i):
                    nonlocal cum
                    cum += weights[i]
                    if i < fill_gate:
                        return
                    want = min(
                        nfill - keep_tail,
                        int(front * nfill * cum / total_w + 1e-9),
                    )
                    while emitted[0] < want:
                        fillers[emitted[0]]()
                        emitted[0] += 1

                slot_idx = 0
                for hp in range(2):
                    po = [
                        ps_o.tile([P, QTILE], f32, tag="o", name=f"po{i}")
                        for i in range(2)
                    ]
                    if diag_first:
                        kt_order = list(range(4 * qt, nkt)) + list(range(4 * qt))
                    else:
                        kt_order = list(range(nkt))
                    pending = []

                    def emit_pv(idx, prab, rq, po=po, hp=hp, nkt=nkt):
                        if "nopv" in probe:
                            if idx == 0:
                                for half in range(2):
                                    nc.vector.memset(po[half][: DK + 1, 0:1], 1.0)
                            return
                        for half in range(2):
                            h = 2 * hp + half
                            nc.tensor.matmul(
                                po[half][: DK + 1, rq:],
                                vones[:, kt_order[idx], h, :],
                                prab[:, half, rq:],
                                start=(idx == 0),
                                stop=(idx == nkt - 1),
                            )

                    for idx, kt in enumerate(kt_order):
                        ktsl = slice(kt * P, (kt + 1) * P)
                        r = kt - 4 * qt
                        rq = max(r, 0) * P  # causally-valid q range start
                        pstile = ps_s.tile([P, 2, QTILE], f32, tag="s", name="ps")
                        if "noscores" in probe:
                            nc.vector.memset(pstile[:, :, rq:rq + 1], 0.0)
                        if "noscores" not in probe:
                            for half in range(2):
                                psl = slice(half * 64, (half + 1) * 64)
                                nc.tensor.matmul(
                                    pstile[:, half, rq:],
                                    ksb[psl, hp, ktsl],
                                    qsb[psl, hp, qt * QTILE + rq:(qt + 1) * QTILE],
                                    start=True,
                                    stop=True,
                                    tile_position=(half * 64, 0),
                                )
                        prab = probsp.tile([P, 2, QTILE], bf16, tag="pr", name="pr")
                        if "noexp" not in probe:
                            nc.scalar.activation(
                                prab[:, :, rq:], pstile[:, :, rq:], Exp, scale=0.125
                            )
                        else:
                            nc.vector.memset(prab[:, :, rq:rq + 1], 1.0)
                        if r >= 0:
                            # mask only the diagonal 128-wide band
                            nc.vector.tensor_mul(
                                prab[:, :, rq:rq + P],
                                prab[:, :, rq:rq + P],
                                maskt[:, None, :].to_broadcast([P, 2, P]),
                            )
                        pending.append((idx, prab, rq))
                        if len(pending) > min(pvdepth, nkt - 2):
                            emit_pv(*pending.pop(0))
                        pace(slot_idx)
                        slot_idx += 1
                        for cl in deferq.pop(slot_idx, ()):
                            cl()
                    for args in pending:
                        emit_pv(*args)
                        # a filler between flushed PVs: each PV may wait on
                        # its exp, so give the PE other work in between
                        if emitted[0] < nfill - keep_tail:
                            fillers[emitted[0]]()
                            emitted[0] += 1

                    def norm_chain(hp=hp, po=po):
                        for half in range(2):
                            rc = yp.tile([1, QTILE], f32, tag="rc", name="rc")
                            nc.vector.reciprocal(rc[:], po[half][DK:DK + 1, :])
                            rb = yp.tile([DK, QTILE], f32, tag="rb", name="rb")
                            nc.gpsimd.partition_broadcast(
                                rb[:], rc[:], channels=DK
                            )
                            nc.vector.tensor_mul(
                                onorm[64 * half:64 * half + 64, hp, :],
                                po[half][0:DK, :],
                                rb[:],
                            )
                        if onorm8 is not None:
                            nc.gpsimd.tensor_copy(
                                onorm8[:, hp, :], onorm[:, hp, :]
                            )
                            nc.gpsimd.tensor_sub(
                                rho8[:, hp, :], onorm[:, hp, :], onorm8[:, hp, :]
                            )

                    if hp == 0:
                        # emit a few slots into hp1 so it doesn't block hp1's
                        # diagonal mask multiplies in the DVE queue
                        deferq.setdefault(nkt + defer_at, []).append(norm_chain)
                    else:
                        deferred = norm_chain

                for cls in deferq.values():  # anything not yet released
                    for cl in cls:
                        cl()
                if final:
                    # norm chain ahead of the held-back fillers' DVE copies,
                    # which then give the PE work during its DVE/Pool latency
                    deferred()
                    deferred = None
                while emitted[0] < nfill:
                    fillers[emitted[0]]()
                    emitted[0] += 1
                return (onorm, onorm8, rho8), deferred

            # software pipeline: round t runs attention(t) interleaved with
            # filler matmul chains. The projections of tile t+1 fill round t;
            # output projections are pushed two rounds late (oproj(t) fills
            # round t+2) because the last round is exp-bound on the
            # Activation engine and needs all the spare PE work it can get.
            def body():
                # tile-0 projections, split in 256-token halves so the first
                # chain starts as soon as the first xt half lands; the second
                # warmup burst bridges until the mt1-3 weights and second xt
                # half arrive (hp0's rope deps, mt 0 and 2, come first so
                # round 0's scores unblock early)
                n_warm2 = opts.get("n_warm2", 6)
                qkv_chain(0, 0, 0, 256, cpy_act=True)
                if n_warm2:
                    wps2 = ps_s.tile([P, 256], f32, tag="s", name="warm2")
                    for i in range(n_warm2):
                        nc.tensor.matmul(
                            wps2[:],
                            scratch[:, 0:P],
                            scratch[:],
                            start=(i == 0),
                            stop=(i == n_warm2 - 1),
                        )
                for mt in (2, 1, 3):
                    qkv_chain(0, mt, 0, 256, cpy_act=True)
                for mt in (0, 2, 1, 3):
                    qkv_chain(0, mt, 256, QTILE, cpy_act=True)
                for tt in range(4):
                    v_chain(0, tt, cpy_act=True)
                onorms = {}
                pre = ()
                # which earlier tiles' output projections fill each round
                oproj_sched = {2: [0, 1], 3: [2]}
                for t in range(NQ):
                    fillers = []
                    if t + 1 < NQ:
                        fillers += phase_b_fillers(t + 1)
                    for qo in oproj_sched.get(t, ()):
                        # the last round's held-back tail chains rotate
                        # through both free psum pools so they aren't
                        # copy-paced through a single 2-slot ring
                        pl = (
                            ((ps_m, "m"), (ps_m, "m"), (ps_s, "s"), (ps_m, "m"),
                             (ps_s, "s"), (ps_m, "m"), (ps_s, "s"), (ps_m, "m"))
                            if t == NQ - 1
                            else ((ps_m, "m"),)
                        )
                        fillers += oproj_fillers(qo, onorms.pop(qo), pools=pl)
                    onorms[t], deferred = run_round(
                        t, fillers, pre=pre,
                        keep_tail=opts.get("keep_tail", 6) if t == NQ - 1 else 2,
                        final=t == NQ - 1,
                    )
                    pre = (deferred,) if deferred is not None else ()
                # final output projection: the kj=0 halves only need hp0's
                # normalized output (ready mid-round), so they run during the
                # hp1 norm chain's DVE/Pool latency; kj=1 + copies follow
                qt = NQ - 1
                onorm = onorms[qt][0]
                qsl = slice(qt * QTILE, (qt + 1) * QTILE)
                ysb = ysbp.tile([P, KO, QTILE], bf16, tag="y", name="ysb")
                pools4 = [(ps_m, "m"), (ps_s, "s")] * 2
                pys = []
                for ot in range(4):
                    pool, ptag = pools4[ot]
                    py = pool.tile([P, QTILE], f32, tag=ptag, name="py")
                    nc.tensor.matmul(
                        py[:], wo[:, 0, ot * P:(ot + 1) * P], onorm[:, 0, :],
                        start=True, stop=False,
                    )
                    pys.append(py)

                def fin_copy(ot):
                    if ot % 2 == 1:
                        nc.scalar.copy(ysb[:, ot, :], pys[ot][:])
                    else:
                        nc.vector.tensor_copy(ysb[:, ot, :], pys[ot][:])

                for ot in range(4):
                    nc.tensor.matmul(
                        pys[ot][:], wo[:, 1, ot * P:(ot + 1) * P],
                        onorm[:, 1, :], start=False, stop=True,
                    )
                    fin_copy(ot)
                    if ot % 2 == 1:  # ship every pair as soon as it's staged
                        nc.sync.dma_start(
                            out_d[:, ot - 1:ot + 1, qsl], ysb[:, ot - 1:ot + 1, :]
                        )
                for ot in range(4, KO):
                    pool, ptag = pools4[ot - 4]
                    py = pool.tile([P, QTILE], f32, tag=ptag, name="py")
                    for kj in range(2):
                        nc.tensor.matmul(
                            py[:], wo[:, kj, ot * P:(ot + 1) * P],
                            onorm[:, kj, :], start=(kj == 0), stop=(kj == 1),
                        )
                    pys.append(py)
                    fin_copy(ot)
                    if ot % 2 == 1:
                        nc.sync.dma_start(
                            out_d[:, ot - 1:ot + 1, qsl], ysb[:, ot - 1:ot + 1, :]
                        )

            if loop:
                with tc.For_i(0, reps, 1):
                    body()
            else:
                for _rep in range(reps):
                    body()
    nc.compile()
    return nc


def _feature_major(rows_x_d, dt=BF):
    """[M, D] (row-major, d = ko*128+ki) -> [P, KO, M] in dtype dt."""
    m = rows_x_d.shape[0]
    return np.ascontiguousarray(
        rows_x_d.T.reshape(KO, P, m).transpose(1, 0, 2)
    ).astype(dt)


def _f8_pair(a):
    """Quantize float32 array to (fp8, fp8 residual)."""
    a8 = a.astype(F8)
    r8 = (a - a8.astype(np.float32)).astype(F8)
    return a8, r8


def _prep_in_maps(x, W_qkv, W_o, token_positions):
    x = np.asarray(x, dtype=np.float32)
    W_qkv = np.asarray(W_qkv, dtype=np.float32)
    W_o = np.asarray(W_o, dtype=np.float32)
    pos = np.asarray(token_positions)

    inv_freq = 1.0 / (
        np.float32(THETA) ** (np.arange(0, DK, 2, dtype=np.float32) / np.float32(DK))
    )
    inv_freq = inv_freq.astype(np.float32)
    freqs = pos.astype(np.float32)[:, :, None] * inv_freq[None, None, :]  # [B,S,32]
    cos = np.cos(freqs).astype(np.float32)
    sin = np.sin(freqs).astype(np.float32)

    jidx = (np.arange(P) % DK) // 2
    sign = np.where(np.arange(P) % 2 == 0, -1.0, 1.0).astype(np.float32)
    # cos/sin tables carry the 1/WSCALE descale of the fp8 QKV psum;
    # combined [P, 2, S] (dim1 = cos, sin) for single-DMA loads
    cs_tab = []
    for b in range(B):
        c = np.ascontiguousarray(cos[b].T[jidx] / WSCALE).astype(BF)
        s = np.ascontiguousarray(sin[b].T[jidx] * sign[:, None] / WSCALE).astype(BF)
        cs_tab.append(np.ascontiguousarray(np.stack([c, s], axis=1)))

    masks = (np.arange(P)[:, None] <= np.arange(P)[None, :]).astype(BF)  # tril^T

    xT2 = []
    for b in range(B):
        fm = np.ascontiguousarray(
            x[b].T.reshape(KO, P, S).transpose(1, 0, 2)
        ).astype(np.float32)
        a8, r8 = _f8_pair(fm)
        # [P, 2, KO, S] -> blocked [P, 2, S//256, KO, 256]
        st = np.stack([a8, r8], axis=1).reshape(P, 2, KO, S // 256, 256)
        xT2.append(np.ascontiguousarray(st.transpose(0, 1, 3, 2, 4)))

    in_maps = []
    for c in range(N_CORES):
        b, hg = divmod(c, 4)
        heads = range(hg * HPC, (hg + 1) * HPC)
        q_rows = np.concatenate([W_qkv[h * DK:(h + 1) * DK] for h in heads])
        k_rows = np.concatenate(
            [W_qkv[D + h * DK:D + (h + 1) * DK] for h in heads]
        )
        v_rows = np.concatenate(
            [W_qkv[2 * D + h * DK:2 * D + (h + 1) * DK] for h in heads]
        )
        wqk_fm = _feature_major(
            np.concatenate([q_rows, k_rows]) * WSCALE, np.float32
        )  # [P, KO, 512]
        # regroup as [P, 4(mt), KO, 128] so each mt slice is one contiguous
        # DMA (the 128-col stationary tiles of the QKV matmul)
        wqk_f = np.ascontiguousarray(
            wqk_fm.reshape(P, KO, 4, P).transpose(0, 2, 1, 3)
        )
        wqk8, wqks8 = _f8_pair(wqk_f)
        wv8, wvs8 = _f8_pair(_feature_major(v_rows * WSCALE, np.float32))
        wo_sub = W_o[:, hg * 256:(hg + 1) * 256]  # [D, 256]
        wo_fm = np.ascontiguousarray(
            wo_sub.T.reshape(2, P, D).transpose(1, 0, 2)
        ).astype(np.float32)  # [P, 2, D]
        # onorm arrives at 8x (ones row = 1/8): bf16 final-tile weights are
        # pre-divided by 8; fp8 weights are pre-multiplied by 8 (net 64x psum)
        wo8, wos8 = _f8_pair(wo_fm * 8.0)
        in_maps.append(
            {
                "xT2": xT2[b],
                "wqk2": np.ascontiguousarray(np.stack([wqk8, wqks8], axis=1)),
                "wv2": np.ascontiguousarray(np.stack([wv8, wvs8], axis=1)),
                "wo": (wo_fm / 8.0).astype(BF),
                "wo2": np.ascontiguousarray(np.stack([wo8, wos8], axis=1)),
                "cossin": cs_tab[b],
                "masks": masks,
            }
        )
    return in_maps


def _get_nc(reps=1, loop=False, probe=(), opts=None):
    key = f"nc{reps}_{loop}_{sorted(probe)}_{sorted((opts or {}).items())}"
    if key not in _CACHE:
        _CACHE[key] = _build_nc(reps, loop, probe, opts)
    return _CACHE[key]


def kernel(x, W_qkv, W_o, token_positions):
    nc = _get_nc()
    in_maps = _prep_in_maps(x, W_qkv, W_o, token_positions)
    res = run_bass_kernel_spmd(nc, in_maps, core_ids=list(range(N_CORES)))
    out = np.zeros((B, S, D), dtype=np.float32)
    for c in range(N_CORES):
        b = c // 4
        # out_t is [P, 8(ot), S] bf16: row d = ot*128 + p of y^T
        yt = np.asarray(res.results[c]["out_t"], dtype=np.float32)
        out[b] += yt.transpose(1, 0, 2).reshape(D, S).T
    return out



# revision 42
# speedup vs baseline: 1.0162x; 1.0162x over previous
"""Causal multi-head self-attention with RoPE on 8 TRN2 NeuronCores.

Sharding: core c handles batch b = c // 4 and heads [4*(c%4), 4*(c%4)+4).
All cores run one SPMD Bass program; per-core behavior comes entirely from
the data (pre-sliced weights, per-batch activations). Each core computes its
4 heads' attention and the partial output projection y^T = W_o_slice^T @ out;
the host sums the 4 partials per batch (the "all-reduce" of the TP split).

Device layout is feature-major throughout: x^T [D, S] feeds QKV as the
moving operand; scores are computed transposed (k on partitions, q free) so
the softmax denominator falls out of a ones-row appended to V in the PV
matmul, and the attention output emerges as out^T [d, q], which is exactly
the moving operand the output projection needs. RoPE is applied on the QKV
PSUM with a pair-swap stream shuffle + host-precomputed cos/sin tables.

The emission is software-pipelined: attention slots for query tile t (whose
per-slot rate is bounded by the Activation engine's exp) are interleaved
with the QKV/V projection chains of tile t+1 and the output projection of
tile t-1, so the Tensor engine always has filler matmuls while exp catches
up. Compute dtype bf16 (fp32 accumulate), f32 in, bf16 partials out (host
accumulates the 4 per-batch partials in f32).

The QKV projection runs in fp8(e4m3) DoubleRow perf mode (0.5 PE cycles per
output row, two 128-deep contraction subtiles per instruction) using a
3-term residual expansion x·W ~= x8·w8 + r8·w8 + x8·s8 with r8 = fp8(x-x8),
s8 = fp8(64W - w8); W is pre-scaled by 64 so its residual clears the e4m3
subnormal floor. All quantization happens host-side; the 1/64 descale is
folded into the cos/sin tables for Q/K and into the V psum-copy multiplier.
"""

import sys

sys.path.insert(0, "/opt/trn_rl_repo")

import numpy as np
import ml_dtypes

import concourse.bass as bass
import concourse.bacc as bacc
import concourse.mybir as mybir
import concourse.tile as tile
from concourse.bass_utils import run_bass_kernel_spmd

B, S, D = 2, 2048, 1024
H, DK = 16, 64
THETA = 10000.0
HPC = 4  # heads per core
P = 128
KO = D // P  # 8 contraction subtiles for the projections
QTILE = 512
NQ = S // QTILE  # 4 query tiles
NKT = S // P  # 16 key-token tiles
N_CORES = 8
BF = ml_dtypes.bfloat16

_PAIRSWAP = [i + 1 if i % 2 == 0 else i - 1 for i in range(32)]

F8 = ml_dtypes.float8_e4m3
WSCALE = 64.0  # host pre-scale on W_qkv so fp8 residuals stay out of subnormals

_CACHE = {}


def _build_nc(reps=1, loop=False, probe=(), opts=None):
    probe = set(probe)
    opts = dict(opts or {})
    vcopy_eng = opts.get("vcopy", "act")
    ycopy_eng = opts.get("ycopy", "dve")
    pvdepth = opts.get("pvdepth", 8)
    probs_bufs = opts.get("probs_bufs", 10)
    rope_bufs = opts.get("rope_bufs", 8)
    yp_bufs = opts.get("yp_bufs", 8)
    on_bufs = opts.get("on_bufs", 3)
    diag_first = opts.get("diag_first", False)
    n_warm = opts.get("n_warm", 12)
    f32 = mybir.dt.float32
    bf16 = mybir.dt.bfloat16
    fp8 = mybir.dt.float8e4
    Exp = mybir.ActivationFunctionType.Exp
    DR = mybir.MatmulPerfMode.DoubleRow

    nc = bacc.Bacc()
    # paired-value/residual tensors are combined on dim1 so each slice is a
    # single DMA (HWDGE generation is a serial 625ns/DMA device); x is blocked
    # by 256-token chunks so every DMA slice is a 2KB contiguous run (descs
    # under 512B pay a 2x DMA latency multiplier)
    NB = S // 256
    xT_d = nc.dram_tensor("xT2", [P, 2, NB, KO, 256], fp8, kind="ExternalInput")
    wqk_d = nc.dram_tensor("wqk2", [P, 2, 4, KO, P], fp8, kind="ExternalInput")
    wv_d = nc.dram_tensor("wv2", [P, 2, KO, HPC * DK], fp8, kind="ExternalInput")
    wo_d = nc.dram_tensor("wo", [P, 2, D], bf16, kind="ExternalInput")
    wo2_d = nc.dram_tensor("wo2", [P, 2, 2, D], fp8, kind="ExternalInput")
    cs_d = nc.dram_tensor("cossin", [P, 2, S], bf16, kind="ExternalInput")
    mask_d = nc.dram_tensor("masks", [P, P], bf16, kind="ExternalInput")
    out_d = nc.dram_tensor("out_t", [P, KO, S], bf16, kind="ExternalOutput")

    with tile.TileContext(nc) as tc:
        with (
            tc.tile_pool(name="const", bufs=1) as cp,
            tc.tile_pool(name="rope", bufs=rope_bufs) as ropep,
            tc.tile_pool(name="probs", bufs=probs_bufs) as probsp,
            tc.tile_pool(name="onp", bufs=on_bufs) as onormp,
            tc.tile_pool(name="on8", bufs=on_bufs) as on8p,
            tc.tile_pool(name="yp", bufs=yp_bufs) as yp,
            tc.tile_pool(name="ysb", bufs=2) as ysbp,
            tc.tile_pool(name="ps_s", bufs=2, space="PSUM") as ps_s,
            tc.tile_pool(name="ps_o", bufs=2, space="PSUM") as ps_o,
            tc.tile_pool(name="ps_m", bufs=2, space="PSUM") as ps_m,
        ):
            xt2 = cp.tile([P, 2, NB, KO, 256], fp8, tag="xT2")
            wqk2 = cp.tile([P, 2, 4, KO, P], fp8, tag="wqk2")
            wv2 = cp.tile([P, 2, KO, HPC * DK], fp8, tag="wv2")
            wo = cp.tile([P, 2, D], bf16, tag="wo")
            wo2 = cp.tile([P, 2, 2, D], fp8, tag="wo2")
            cossin = cp.tile([P, 2, S], bf16, tag="cossin")
            maskt = cp.tile([P, P], bf16, tag="mask")
            vones = cp.tile([P, NKT, HPC, DK + 1], bf16, tag="vones")
            qsb = cp.tile([P, 2, S], bf16, tag="qsb")
            ksb = cp.tile([P, 2, S], bf16, tag="ksb")
            scratch = cp.tile([P, 256], bf16, tag="scr")

            # PE pre-ramp: the p-state model runs matmuls at reduced clock
            # until the engine has been continuously busy ~3us. Warm it up on
            # scratch data while the first input DMAs are in flight; the ramp
            # then carries into the real matmul stream with no idle gap.
            # the ones row is 1/8 so onorm comes out as 8*att, matching the
            # x8 pre-scale of the fp8 output-projection weights
            nc.gpsimd.memset(scratch[:], 0.0)
            nc.vector.memset(vones[:, :, :, DK:DK + 1], 0.125)
            if n_warm:
                wps = ps_s.tile([P, 256], f32, tag="s", name="warm")
                for i in range(n_warm):
                    nc.tensor.matmul(
                        wps[:],
                        scratch[:, 0:P],
                        scratch[:],
                        start=(i == 0),
                        stop=(i == n_warm - 1),
                    )

            # input DMAs, all on the SP queue. Each DMA costs one 625ns HWDGE
            # slot and its transfer serializes on the single DMA device, so
            # order = need-time: the first QKV chain's operands in fine grain
            # (the chain starts as soon as ko 0 lands and is paced by the
            # rest), then everything else batched coarse.
            nc.sync.dma_start(wqk2[:, :, 0], wqk_d[:, :, 0])
            nc.sync.dma_start(xt2[:, :, 0], xT_d[:, :, 0])
            nc.sync.dma_start(wqk2[:, :, 1:4], wqk_d[:, :, 1:4])
            nc.sync.dma_start(cossin[:, :, 0:QTILE], cs_d[:, :, 0:QTILE])
            nc.sync.dma_start(xt2[:, :, 1], xT_d[:, :, 1])
            nc.sync.dma_start(maskt[:], mask_d[:])
            nc.sync.dma_start(wv2[:], wv_d[:])
            for nt in range(1, NQ):
                sl = slice(nt * QTILE, (nt + 1) * QTILE)
                nc.sync.dma_start(xt2[:, :, 2 * nt:2 * nt + 2], xT_d[:, :, 2 * nt:2 * nt + 2])
                nc.sync.dma_start(cossin[:, :, sl], cs_d[:, :, sl])
                if nt == 2:
                    nc.sync.dma_start(wo2[:], wo2_d[:])
                if nt == 3:
                    nc.sync.dma_start(wo[:], wo_d[:])

            def qkv_chain(nt, mt, lo=0, hi=QTILE, cpy_act=False):
                """One 128-col QKV chain + psum copy + rope to qsb/ksb.

                3-term fp8 DoubleRow: x8·w8 + r8·w8 + x8·s8 in 256-col
                chunks (DoubleRow moving free = 2N <= 512)."""
                nsl = slice(nt * QTILE + lo, nt * QTILE + hi)
                w = hi - lo
                pqk = ps_m.tile([P, w], f32, tag="m", name="pqk")
                for c0 in range(0, w, 256):
                    cw = min(256, w - c0)
                    blk = (nt * QTILE + lo + c0) // 256
                    ni = 0
                    for jw, jx in ((0, 0), (0, 1), (1, 0)):
                        for t in range(KO // 2):
                            nc.tensor.matmul(
                                pqk[:, c0:c0 + cw],
                                wqk2[:, jw, mt, 2 * t:2 * t + 2, :],
                                xt2[:, jx, blk, 2 * t:2 * t + 2, 0:cw],
                                start=(ni == 0),
                                stop=(ni == 3 * (KO // 2) - 1),
                                perf_mode=DR,
                            )
                            ni += 1
                cpy = ropep.tile([P, w], bf16, tag="cpy", name="cpy")
                if cpy_act:  # prologue: the exp stream hasn't started yet
                    nc.scalar.copy(cpy[:], pqk[:])
                else:
                    nc.vector.tensor_copy(cpy[:], pqk[:])
                t0 = ropep.tile([P, w], bf16, tag="t0", name="t0")
                sw = ropep.tile([P, w], bf16, tag="sw", name="sw")
                u0 = ropep.tile([P, w], bf16, tag="u0", name="u0")
                nc.vector.tensor_mul(t0[:], cpy[:], cossin[:, 0, nsl])
                nc.vector.stream_shuffle(sw[:], cpy[:], _PAIRSWAP)
                nc.vector.tensor_mul(u0[:], sw[:], cossin[:, 1, nsl])
                dst = qsb if mt < 2 else ksb
                nc.vector.tensor_add(dst[:, mt % 2, nsl], t0[:], u0[:])

            def v_chain(nt, tt, cpy_act=False):
                """V projection for one 128-token subtile (3-term fp8 DR).
                The psum holds 64·v; the copy descales by 1/64."""
                kt = nt * 4 + tt
                blk, off = kt // 2, (kt % 2) * P
                pv = ps_m.tile([P, HPC, DK], f32, tag="m", name="pv")
                ni = 0
                for jx, jw in ((0, 0), (1, 0), (0, 1)):
                    for t in range(KO // 2):
                        nc.tensor.matmul(
                            pv[:],
                            xt2[:, jx, blk, 2 * t:2 * t + 2, off:off + P],
                            wv2[:, jw, 2 * t:2 * t + 2, :],
                            start=(ni == 0),
                            stop=(ni == 3 * (KO // 2) - 1),
                            perf_mode=DR,
                        )
                        ni += 1
                if vcopy_eng == "dve" and not cpy_act:
                    nc.vector.tensor_scalar_mul(
                        vones[:, kt, :, 0:DK], pv[:], 1.0 / WSCALE
                    )
                else:
                    nc.scalar.mul(vones[:, kt, :, 0:DK], pv[:], 1.0 / WSCALE)

            def phase_b_fillers(nt):
                return [lambda mt=mt: qkv_chain(nt, mt) for mt in range(4)] + [
                    lambda tt=tt: v_chain(nt, tt) for tt in range(4)
                ]

            def oproj_fillers(qt, onorm_pack, pools=((ps_m, "m"),)):
                """Output projection chains for query tile qt (3-term fp8
                DoubleRow; psum lands at 64x so copies descale by 1/64).
                The later tiles run near the kernel tail where the exp
                stream has ended, so their copies alternate onto the idle
                Activation engine and their output DMAs are split to keep
                the last transfer small."""
                onorm, onorm8, rho8 = onorm_pack
                qsl = slice(qt * QTILE, (qt + 1) * QTILE)
                late = qt >= NQ - 2
                last = qt == NQ - 1
                ysb = ysbp.tile([P, KO, QTILE], bf16, tag="y", name="ysb")

                def ochain(ot):
                    pool, ptag = pools[ot % len(pools)]
                    py = pool.tile([P, QTILE], f32, tag=ptag, name="py")
                    otsl = slice(ot * P, (ot + 1) * P)
                    for c0 in (0, 256):
                        for ni, (j, mv) in enumerate(
                            ((0, onorm8), (0, rho8), (1, onorm8))
                        ):
                            nc.tensor.matmul(
                                py[:, c0:c0 + 256],
                                wo2[:, j, 0:2, otsl],
                                mv[:, 0:2, c0:c0 + 256],
                                start=(ni == 0),
                                stop=(ni == 2),
                                perf_mode=DR,
                            )
                    if "noy" in probe:
                        return
                    # Activation copies only where the exp stream has ended
                    # (the held-back tail chains); mid-round copies stay on
                    # DVE so they don't stretch the exp-bound final round
                    use_act = ycopy_eng == "act" or (qt == NQ - 2 and ot >= 2)
                    if use_act:
                        nc.scalar.mul(ysb[:, ot, :], py[:], 1.0 / WSCALE)
                    else:
                        nc.vector.tensor_scalar_mul(
                            ysb[:, ot, :], py[:], 1.0 / WSCALE
                        )
                    # split the late tiles' output DMAs so the tail after the
                    # last matmul is a small transfer, not a whole tile
                    if late and ot == 3:
                        nc.sync.dma_start(out_d[:, 0:4, qsl], ysb[:, 0:4, :])
                    elif last and ot == 5:
                        nc.sync.dma_start(out_d[:, 4:6, qsl], ysb[:, 4:6, :])
                    elif last and ot == 6:
                        nc.sync.dma_start(out_d[:, 6:7, qsl], ysb[:, 6:7, :])
                    elif last and ot == KO - 1:
                        nc.sync.dma_start(out_d[:, 7:8, qsl], ysb[:, 7:8, :])
                    elif not last and ot == KO - 1:
                        if late:
                            nc.sync.dma_start(out_d[:, 4:8, qsl], ysb[:, 4:8, :])
                        else:
                            nc.sync.dma_start(out_d[:, :, qsl], ysb[:])

                return [lambda ot=ot: ochain(ot) for ot in range(KO)]

            def run_round(qt, fillers, pre=(), defer_at=4, keep_tail=2,
                          final=False):
                """Attention for query tile qt, interleaved with fillers.

                ``pre`` holds deferred closures (the previous round's softmax
                normalization chains): they are emitted a few slots in, so
                they don't head-of-line-block the DVE queue ahead of this
                round's first diagonal mask multiplies. This round's own norm
                chains are returned as closures for the next round (the last
                round emits them inline). Filler pacing is weighted by each
                slot's PE deficit: a diagonal slot has little matmul work but
                a full exp, so it gets more filler coverage.
                """
                onorm = onormp.tile([P, 2, QTILE], bf16, tag="on", name="onorm")
                if qt < NQ - 1:
                    # fp8 quantized onorm + residual for the fp8 output
                    # projection (produced on the otherwise-idle Pool engine)
                    onorm8 = on8p.tile([P, 2, QTILE], fp8, tag="o8", name="on8")
                    rho8 = on8p.tile([P, 2, QTILE], fp8, tag="r8", name="rho8")
                else:
                    onorm8 = rho8 = None
                nkt = 4 * (qt + 1)
                nfill = len(fillers)
                emitted = [0]
                # deferred closures keyed by the slot index that releases them
                deferq = {defer_at: list(pre)}

                # per-slot PE deficit weight: full-slot ACT time is roughly
                # constant, PE slot work scales with the causal width
                weights = []
                for hp in range(2):
                    for kt in (range(4 * qt, nkt) if diag_first else range(nkt)):
                        r = kt - 4 * qt
                        rq = max(r, 0) * P
                        weights.append(1.0 + 3.0 * rq / QTILE)
                    if diag_first:
                        weights.extend([1.0] * (4 * qt))
                total_w = sum(weights)
                nslots = len(weights)
                cum = 0.0

                front = 1.5 if final else 1.0
                # fillers may consume onorm written by the deferred norm
                # closures in ``pre`` — hold them until those are emitted
                fill_gate = defer_at if pre else 0

                def pace(i):
                    nonlocal cum
                    cum += weights[i]
                    if i < fill_gate:
                        return
                    want = min(
                        nfill - keep_tail,
                        int(front * nfill * cum / total_w + 1e-9),
                    )
                    while emitted[0] < want:
                        fillers[emitted[0]]()
                        emitted[0] += 1

                slot_idx = 0
                for hp in range(2):
                    po = [
                        ps_o.tile([P, QTILE], f32, tag="o", name=f"po{i}")
                        for i in range(2)
                    ]
                    if diag_first:
                        kt_order = list(range(4 * qt, nkt)) + list(range(4 * qt))
                    else:
                        kt_order = list(range(nkt))
                    pending = []

                    def emit_pv(idx, prab, rq, po=po, hp=hp, nkt=nkt):
                        if "nopv" in probe:
                            if idx == 0:
                                for half in range(2):
                                    nc.vector.memset(po[half][: DK + 1, 0:1], 1.0)
                            return
                        for half in range(2):
                            h = 2 * hp + half
                            nc.tensor.matmul(
                                po[half][: DK + 1, rq:],
                                vones[:, kt_order[idx], h, :],
                                prab[:, half, rq:],
                                start=(idx == 0),
                                stop=(idx == nkt - 1),
                            )

                    for idx, kt in enumerate(kt_order):
                        ktsl = slice(kt * P, (kt + 1) * P)
                        r = kt - 4 * qt
                        rq = max(r, 0) * P  # causally-valid q range start
                        pstile = ps_s.tile([P, 2, QTILE], f32, tag="s", name="ps")
                        if "noscores" in probe:
                            nc.vector.memset(pstile[:, :, rq:rq + 1], 0.0)
                        if "noscores" not in probe:
                            for half in range(2):
                                psl = slice(half * 64, (half + 1) * 64)
                                nc.tensor.matmul(
                                    pstile[:, half, rq:],
                                    ksb[psl, hp, ktsl],
                                    qsb[psl, hp, qt * QTILE + rq:(qt + 1) * QTILE],
                                    start=True,
                                    stop=True,
                                    tile_position=(half * 64, 0),
                                )
                        prab = probsp.tile([P, 2, QTILE], bf16, tag="pr", name="pr")
                        if "noexp" not in probe:
                            nc.scalar.activation(
                                prab[:, :, rq:], pstile[:, :, rq:], Exp, scale=0.125
                            )
                        else:
                            nc.vector.memset(prab[:, :, rq:rq + 1], 1.0)
                        if r >= 0:
                            # mask only the diagonal 128-wide band
                            nc.vector.tensor_mul(
                                prab[:, :, rq:rq + P],
                                prab[:, :, rq:rq + P],
                                maskt[:, None, :].to_broadcast([P, 2, P]),
                            )
                        pending.append((idx, prab, rq))
                        if len(pending) > min(pvdepth, nkt - 2):
                            emit_pv(*pending.pop(0))
                        pace(slot_idx)
                        slot_idx += 1
                        for cl in deferq.pop(slot_idx, ()):
                            cl()
                    for args in pending:
                        emit_pv(*args)
                        # a filler between flushed PVs: each PV may wait on
                        # its exp, so give the PE other work in between
                        if emitted[0] < nfill - keep_tail:
                            fillers[emitted[0]]()
                            emitted[0] += 1

                    def norm_chain(hp=hp, po=po):
                        for half in range(2):
                            rc = yp.tile([1, QTILE], f32, tag="rc", name="rc")
                            nc.vector.reciprocal(rc[:], po[half][DK:DK + 1, :])
                            rb = yp.tile([DK, QTILE], f32, tag="rb", name="rb")
                            nc.gpsimd.partition_broadcast(
                                rb[:], rc[:], channels=DK
                            )
                            nc.vector.tensor_mul(
                                onorm[64 * half:64 * half + 64, hp, :],
                                po[half][0:DK, :],
                                rb[:],
                            )
                        if onorm8 is not None:
                            nc.gpsimd.tensor_copy(
                                onorm8[:, hp, :], onorm[:, hp, :]
                            )
                            nc.gpsimd.tensor_sub(
                                rho8[:, hp, :], onorm[:, hp, :], onorm8[:, hp, :]
                            )

                    if hp == 0:
                        # emit a few slots into hp1 so it doesn't block hp1's
                        # diagonal mask multiplies in the DVE queue
                        deferq.setdefault(nkt + defer_at, []).append(norm_chain)
                    else:
                        deferred = norm_chain

                for cls in deferq.values():  # anything not yet released
                    for cl in cls:
                        cl()
                if final:
                    # norm chain ahead of the held-back fillers' DVE copies,
                    # which then give the PE work during its DVE/Pool latency
                    deferred()
                    deferred = None
                while emitted[0] < nfill:
                    fillers[emitted[0]]()
                    emitted[0] += 1
                return (onorm, onorm8, rho8), deferred

            # software pipeline: round t runs attention(t) interleaved with
            # filler matmul chains. The projections of tile t+1 fill round t;
            # output projections are pushed two rounds late (oproj(t) fills
            # round t+2) because the last round is exp-bound on the
            # Activation engine and needs all the spare PE work it can get.
            def body():
                # tile-0 projections, split in 256-token halves so the first
                # chain starts as soon as the first xt half lands; the second
                # warmup burst bridges until the mt1-3 weights and second xt
                # half arrive (hp0's rope deps, mt 0 and 2, come first so
                # round 0's scores unblock early)
                n_warm2 = opts.get("n_warm2", 6)
                qkv_chain(0, 0, 0, 256, cpy_act=True)
                if n_warm2:
                    wps2 = ps_s.tile([P, 256], f32, tag="s", name="warm2")
                    for i in range(n_warm2):
                        nc.tensor.matmul(
                            wps2[:],
                            scratch[:, 0:P],
                            scratch[:],
                            start=(i == 0),
                            stop=(i == n_warm2 - 1),
                        )
                for mt in (2, 1, 3):
                    qkv_chain(0, mt, 0, 256, cpy_act=True)
                for mt in (0, 2, 1, 3):
                    qkv_chain(0, mt, 256, QTILE, cpy_act=True)
                for tt in range(4):
                    v_chain(0, tt, cpy_act=True)
                onorms = {}
                pre = ()
                # which earlier tiles' output projections fill each round
                oproj_sched = opts.get("oproj_sched", {2: (0, 1), 3: (2,)})
                for t in range(NQ):
                    fillers = []
                    if t + 1 < NQ:
                        fillers += phase_b_fillers(t + 1)
                    for qo in oproj_sched.get(t, ()):
                        # the last round's held-back tail chains rotate
                        # through both free psum pools so they aren't
                        # copy-paced through a single 2-slot ring
                        pl = (
                            ((ps_m, "m"), (ps_m, "m"), (ps_s, "s"), (ps_m, "m"),
                             (ps_s, "s"), (ps_m, "m"), (ps_s, "s"), (ps_m, "m"))
                            if t == NQ - 1
                            else ((ps_m, "m"),)
                        )
                        fillers += oproj_fillers(qo, onorms.pop(qo), pools=pl)
                    onorms[t], deferred = run_round(
                        t, fillers, pre=pre,
                        keep_tail=opts.get("keep_tail", 8) if t == NQ - 1 else 2,
                        final=t == NQ - 1,
                    )
                    pre = (deferred,) if deferred is not None else ()
                # final output projection: the kj=0 halves only need hp0's
                # normalized output (ready mid-round), so they run during the
                # hp1 norm chain's DVE/Pool latency; kj=1 + copies follow
                qt = NQ - 1
                onorm = onorms[qt][0]
                qsl = slice(qt * QTILE, (qt + 1) * QTILE)
                ysb = ysbp.tile([P, KO, QTILE], bf16, tag="y", name="ysb")
                pools4 = [(ps_m, "m"), (ps_s, "s")] * 2
                pys = []
                for ot in range(4):
                    pool, ptag = pools4[ot]
                    py = pool.tile([P, QTILE], f32, tag=ptag, name="py")
                    nc.tensor.matmul(
                        py[:], wo[:, 0, ot * P:(ot + 1) * P], onorm[:, 0, :],
                        start=True, stop=False,
                    )
                    pys.append(py)

                def fin_copy(ot):
                    if ot % 2 == 1:
                        nc.scalar.copy(ysb[:, ot, :], pys[ot][:])
                    else:
                        nc.vector.tensor_copy(ysb[:, ot, :], pys[ot][:])

                for ot in range(4):
                    nc.tensor.matmul(
                        pys[ot][:], wo[:, 1, ot * P:(ot + 1) * P],
                        onorm[:, 1, :], start=False, stop=True,
                    )
                    fin_copy(ot)
                    if ot % 2 == 1:  # ship every pair as soon as it's staged
                        nc.sync.dma_start(
                            out_d[:, ot - 1:ot + 1, qsl], ysb[:, ot - 1:ot + 1, :]
                        )
                for ot in range(4, KO):
                    pool, ptag = pools4[ot - 4]
                    py = pool.tile([P, QTILE], f32, tag=ptag, name="py")
                    for kj in range(2):
                        nc.tensor.matmul(
                            py[:], wo[:, kj, ot * P:(ot + 1) * P],
                            onorm[:, kj, :], start=(kj == 0), stop=(kj == 1),
                        )
                    pys.append(py)
                    fin_copy(ot)
                    if ot % 2 == 1:
                        nc.sync.dma_start(
                            out_d[:, ot - 1:ot + 1, qsl], ysb[:, ot - 1:ot + 1, :]
                        )

            if loop:
                with tc.For_i(0, reps, 1):
                    body()
            else:
                for _rep in range(reps):
                    body()
    nc.compile()
    return nc


def _feature_major(rows_x_d, dt=BF):
    """[M, D] (row-major, d = ko*128+ki) -> [P, KO, M] in dtype dt."""
    m = rows_x_d.shape[0]
    return np.ascontiguousarray(
        rows_x_d.T.reshape(KO, P, m).transpose(1, 0, 2)
    ).astype(dt)


def _f8_pair(a):
    """Quantize float32 array to (fp8, fp8 residual)."""
    a8 = a.astype(F8)
    r8 = (a - a8.astype(np.float32)).astype(F8)
    return a8, r8


def _prep_in_maps(x, W_qkv, W_o, token_positions):
    x = np.asarray(x, dtype=np.float32)
    W_qkv = np.asarray(W_qkv, dtype=np.float32)
    W_o = np.asarray(W_o, dtype=np.float32)
    pos = np.asarray(token_positions)

    inv_freq = 1.0 / (
        np.float32(THETA) ** (np.arange(0, DK, 2, dtype=np.float32) / np.float32(DK))
    )
    inv_freq = inv_freq.astype(np.float32)
    freqs = pos.astype(np.float32)[:, :, None] * inv_freq[None, None, :]  # [B,S,32]
    cos = np.cos(freqs).astype(np.float32)
    sin = np.sin(freqs).astype(np.float32)

    jidx = (np.arange(P) % DK) // 2
    sign = np.where(np.arange(P) % 2 == 0, -1.0, 1.0).astype(np.float32)
    # cos/sin tables carry the 1/WSCALE descale of the fp8 QKV psum;
    # combined [P, 2, S] (dim1 = cos, sin) for single-DMA loads
    cs_tab = []
    for b in range(B):
        c = np.ascontiguousarray(cos[b].T[jidx] / WSCALE).astype(BF)
        s = np.ascontiguousarray(sin[b].T[jidx] * sign[:, None] / WSCALE).astype(BF)
        cs_tab.append(np.ascontiguousarray(np.stack([c, s], axis=1)))

    masks = (np.arange(P)[:, None] <= np.arange(P)[None, :]).astype(BF)  # tril^T

    xT2 = []
    for b in range(B):
        fm = np.ascontiguousarray(
            x[b].T.reshape(KO, P, S).transpose(1, 0, 2)
        ).astype(np.float32)
        a8, r8 = _f8_pair(fm)
        # [P, 2, KO, S] -> blocked [P, 2, S//256, KO, 256]
        st = np.stack([a8, r8], axis=1).reshape(P, 2, KO, S // 256, 256)
        xT2.append(np.ascontiguousarray(st.transpose(0, 1, 3, 2, 4)))

    in_maps = []
    for c in range(N_CORES):
        b, hg = divmod(c, 4)
        heads = range(hg * HPC, (hg + 1) * HPC)
        q_rows = np.concatenate([W_qkv[h * DK:(h + 1) * DK] for h in heads])
        k_rows = np.concatenate(
            [W_qkv[D + h * DK:D + (h + 1) * DK] for h in heads]
        )
        v_rows = np.concatenate(
            [W_qkv[2 * D + h * DK:2 * D + (h + 1) * DK] for h in heads]
        )
        wqk_fm = _feature_major(
            np.concatenate([q_rows, k_rows]) * WSCALE, np.float32
        )  # [P, KO, 512]
        # regroup as [P, 4(mt), KO, 128] so each mt slice is one contiguous
        # DMA (the 128-col stationary tiles of the QKV matmul)
        wqk_f = np.ascontiguousarray(
            wqk_fm.reshape(P, KO, 4, P).transpose(0, 2, 1, 3)
        )
        wqk8, wqks8 = _f8_pair(wqk_f)
        wv8, wvs8 = _f8_pair(_feature_major(v_rows * WSCALE, np.float32))
        wo_sub = W_o[:, hg * 256:(hg + 1) * 256]  # [D, 256]
        wo_fm = np.ascontiguousarray(
            wo_sub.T.reshape(2, P, D).transpose(1, 0, 2)
        ).astype(np.float32)  # [P, 2, D]
        # onorm arrives at 8x (ones row = 1/8): bf16 final-tile weights are
        # pre-divided by 8; fp8 weights are pre-multiplied by 8 (net 64x psum)
        wo8, wos8 = _f8_pair(wo_fm * 8.0)
        in_maps.append(
            {
                "xT2": xT2[b],
                "wqk2": np.ascontiguousarray(np.stack([wqk8, wqks8], axis=1)),
                "wv2": np.ascontiguousarray(np.stack([wv8, wvs8], axis=1)),
                "wo": (wo_fm / 8.0).astype(BF),
                "wo2": np.ascontiguousarray(np.stack([wo8, wos8], axis=1)),
                "cossin": cs_tab[b],
                "masks": masks,
            }
        )
    return in_maps


def _get_nc(reps=1, loop=False, probe=(), opts=None):
    key = f"nc{reps}_{loop}_{sorted(probe)}_{sorted((opts or {}).items())}"
    if key not in _CACHE:
        _CACHE[key] = _build_nc(reps, loop, probe, opts)
    return _CACHE[key]


def kernel(x, W_qkv, W_o, token_positions):
    nc = _get_nc()
    in_maps = _prep_in_maps(x, W_qkv, W_o, token_positions)
    res = run_bass_kernel_spmd(nc, in_maps, core_ids=list(range(N_CORES)))
    out = np.zeros((B, S, D), dtype=np.float32)
    for c in range(N_CORES):
        b = c // 4
        # out_t is [P, 8(ot), S] bf16: row d = ot*128 + p of y^T
        yt = np.asarray(res.results[c]["out_t"], dtype=np.float32)
        out[b] += yt.transpose(1, 0, 2).reshape(D, S).T
    return out



# revision 51
# speedup vs baseline: 1.0298x; 1.0134x over previous
"""Causal multi-head self-attention with RoPE on 8 TRN2 NeuronCores.

Sharding: core c handles batch b = c // 4 and heads [4*(c%4), 4*(c%4)+4).
All cores run one SPMD Bass program; per-core behavior comes entirely from
the data (pre-sliced weights, per-batch activations). Each core computes its
4 heads' attention and the partial output projection y^T = W_o_slice^T @ out;
the host sums the 4 partials per batch (the "all-reduce" of the TP split).

Device layout is feature-major throughout: x^T [D, S] feeds QKV as the
moving operand; scores are computed transposed (k on partitions, q free) so
the softmax denominator falls out of a ones-row appended to V in the PV
matmul, and the attention output emerges as out^T [d, q], which is exactly
the moving operand the output projection needs. RoPE is applied on the QKV
PSUM with a pair-swap stream shuffle + host-precomputed cos/sin tables.

The emission is software-pipelined: attention slots for query tile t (whose
per-slot rate is bounded by the Activation engine's exp) are interleaved
with the QKV/V projection chains of tile t+1 and the output projection of
tile t-1, so the Tensor engine always has filler matmuls while exp catches
up. Compute dtype bf16 (fp32 accumulate), f32 in, bf16 partials out (host
accumulates the 4 per-batch partials in f32).

The QKV projection runs in fp8(e4m3) DoubleRow perf mode (0.5 PE cycles per
output row, two 128-deep contraction subtiles per instruction) using a
3-term residual expansion x·W ~= x8·w8 + r8·w8 + x8·s8 with r8 = fp8(x-x8),
s8 = fp8(64W - w8); W is pre-scaled by 64 so its residual clears the e4m3
subnormal floor. All quantization happens host-side; the 1/64 descale is
folded into the cos/sin tables for Q/K and into the V psum-copy multiplier.
"""

import sys

sys.path.insert(0, "/opt/trn_rl_repo")

import numpy as np
import ml_dtypes

import concourse.bass as bass
import concourse.bacc as bacc
import concourse.mybir as mybir
import concourse.tile as tile
from concourse.bass_utils import run_bass_kernel_spmd

B, S, D = 2, 2048, 1024
H, DK = 16, 64
THETA = 10000.0
HPC = 4  # heads per core
P = 128
KO = D // P  # 8 contraction subtiles for the projections
QTILE = 512
NQ = S // QTILE  # 4 query tiles
NKT = S // P  # 16 key-token tiles
N_CORES = 8
BF = ml_dtypes.bfloat16

_PAIRSWAP = [i + 1 if i % 2 == 0 else i - 1 for i in range(32)]

F8 = ml_dtypes.float8_e4m3
WSCALE = 64.0  # host pre-scale on W_qkv so fp8 residuals stay out of subnormals

_CACHE = {}


def _build_nc(reps=1, loop=False, probe=(), opts=None):
    probe = set(probe)
    opts = dict(opts or {})
    vcopy_eng = opts.get("vcopy", "act")
    ycopy_eng = opts.get("ycopy", "dve")
    pvdepth = opts.get("pvdepth", 8)
    probs_bufs = opts.get("probs_bufs", 10)
    rope_bufs = opts.get("rope_bufs", 8)
    yp_bufs = opts.get("yp_bufs", 8)
    on_bufs = opts.get("on_bufs", 3)
    diag_first = opts.get("diag_first", False)
    n_warm = opts.get("n_warm", 12)
    f32 = mybir.dt.float32
    bf16 = mybir.dt.bfloat16
    fp8 = mybir.dt.float8e4
    Exp = mybir.ActivationFunctionType.Exp
    DR = mybir.MatmulPerfMode.DoubleRow

    nc = bacc.Bacc()
    # paired-value/residual tensors are combined on dim1 so each slice is a
    # single DMA (HWDGE generation is a serial 625ns/DMA device); x is blocked
    # by 256-token chunks so every DMA slice is a 2KB contiguous run (descs
    # under 512B pay a 2x DMA latency multiplier)
    NB = S // 256
    xT_d = nc.dram_tensor("xT2", [P, 2, NB, KO, 256], fp8, kind="ExternalInput")
    wqk_d = nc.dram_tensor("wqk2", [P, 2, 4, KO, P], fp8, kind="ExternalInput")
    wv_d = nc.dram_tensor("wv2", [P, 2, KO, HPC * DK], fp8, kind="ExternalInput")
    wo_d = nc.dram_tensor("wo", [P, 2, D], bf16, kind="ExternalInput")
    wo2_d = nc.dram_tensor("wo2", [P, 2, 2, D], fp8, kind="ExternalInput")
    cs_d = nc.dram_tensor("cossin", [P, 2, S], bf16, kind="ExternalInput")
    mask_d = nc.dram_tensor("masks", [P, P], bf16, kind="ExternalInput")
    out_d = nc.dram_tensor("out_t", [P, KO, S], bf16, kind="ExternalOutput")

    with tile.TileContext(nc) as tc:
        with (
            tc.tile_pool(name="const", bufs=1) as cp,
            tc.tile_pool(name="rope", bufs=rope_bufs) as ropep,
            tc.tile_pool(name="probs", bufs=probs_bufs) as probsp,
            tc.tile_pool(name="onp", bufs=on_bufs) as onormp,
            tc.tile_pool(name="on8", bufs=on_bufs) as on8p,
            tc.tile_pool(name="yp", bufs=yp_bufs) as yp,
            tc.tile_pool(name="ysb", bufs=2) as ysbp,
            tc.tile_pool(name="ps_s", bufs=2, space="PSUM") as ps_s,
            tc.tile_pool(name="ps_o", bufs=2, space="PSUM") as ps_o,
            tc.tile_pool(name="ps_m", bufs=2, space="PSUM") as ps_m,
        ):
            xt2 = cp.tile([P, 2, NB, KO, 256], fp8, tag="xT2")
            wqk2 = cp.tile([P, 2, 4, KO, P], fp8, tag="wqk2")
            wv2 = cp.tile([P, 2, KO, HPC * DK], fp8, tag="wv2")
            wo = cp.tile([P, 2, D], bf16, tag="wo")
            wo2 = cp.tile([P, 2, 2, D], fp8, tag="wo2")
            cossin = cp.tile([P, 2, S], bf16, tag="cossin")
            maskt = cp.tile([P, P], bf16, tag="mask")
            vones = cp.tile([P, NKT, HPC, DK + 1], bf16, tag="vones")
            qsb = cp.tile([P, 2, S], bf16, tag="qsb")
            ksb = cp.tile([P, 2, S], bf16, tag="ksb")
            # raw (non-tile) sbuf tensor: dependency-free so the warmup
            # matmuls can start immediately without waiting on a memset
            scratch = nc.alloc_sbuf_tensor("warm_scr", [P, 256], bf16)

            # PE pre-ramp: the p-state model runs matmuls at reduced clock
            # until the engine has been continuously busy ~3us. Warm it up on
            # scratch data while the first input DMAs are in flight; the ramp
            # then carries into the real matmul stream with no idle gap.
            # the ones row is 1/8 so onorm comes out as 8*att, matching the
            # x8 pre-scale of the fp8 output-projection weights.
            # scratch is intentionally never initialized: the warmup matmuls
            # only exist to hold the PE p-state ramp, their psum is never
            # read, and skipping the memset lets the PE start immediately.
            nc.vector.memset(vones[:, :, :, DK:DK + 1], 0.125)
            if n_warm:
                wps = ps_s.tile([P, 256], f32, tag="s", name="warm")
                for i in range(n_warm):
                    nc.tensor.matmul(
                        wps[:],
                        scratch[:, 0:P],
                        scratch[:],
                        start=(i == 0),
                        stop=(i == n_warm - 1),
                    )

            # input DMAs, all on the SP queue. Each DMA costs one 625ns HWDGE
            # slot and its transfer serializes on the single DMA device, so
            # order = need-time: the first QKV chain's operands in fine grain
            # (the chain starts as soon as ko 0 lands and is paced by the
            # rest), then everything else batched coarse.
            nc.sync.dma_start(wqk2[:, :, 0], wqk_d[:, :, 0])
            nc.sync.dma_start(xt2[:, :, 0], xT_d[:, :, 0])
            nc.sync.dma_start(wqk2[:, :, 1:4], wqk_d[:, :, 1:4])
            nc.sync.dma_start(cossin[:, :, 0:QTILE], cs_d[:, :, 0:QTILE])
            nc.sync.dma_start(xt2[:, :, 1], xT_d[:, :, 1])
            nc.sync.dma_start(maskt[:], mask_d[:])
            nc.sync.dma_start(wv2[:], wv_d[:])
            for nt in range(1, NQ):
                sl = slice(nt * QTILE, (nt + 1) * QTILE)
                nc.sync.dma_start(xt2[:, :, 2 * nt:2 * nt + 2], xT_d[:, :, 2 * nt:2 * nt + 2])
                nc.sync.dma_start(cossin[:, :, sl], cs_d[:, :, sl])
                if nt == 2:
                    nc.sync.dma_start(wo2[:], wo2_d[:])
                if nt == 3:
                    nc.sync.dma_start(wo[:], wo_d[:])

            def qkv_chain(nt, mt, lo=0, hi=QTILE, cpy_act=False):
                """One 128-col QKV chain + psum copy + rope to qsb/ksb.

                3-term fp8 DoubleRow: x8·w8 + r8·w8 + x8·s8 in 256-col
                chunks (DoubleRow moving free = 2N <= 512)."""
                nsl = slice(nt * QTILE + lo, nt * QTILE + hi)
                w = hi - lo
                pqk = ps_m.tile([P, w], f32, tag="m", name="pqk")
                for c0 in range(0, w, 256):
                    cw = min(256, w - c0)
                    blk = (nt * QTILE + lo + c0) // 256
                    ni = 0
                    for jw, jx in ((0, 0), (0, 1), (1, 0)):
                        for t in range(KO // 2):
                            nc.tensor.matmul(
                                pqk[:, c0:c0 + cw],
                                wqk2[:, jw, mt, 2 * t:2 * t + 2, :],
                                xt2[:, jx, blk, 2 * t:2 * t + 2, 0:cw],
                                start=(ni == 0),
                                stop=(ni == 3 * (KO // 2) - 1),
                                perf_mode=DR,
                            )
                            ni += 1
                cpy = ropep.tile([P, w], bf16, tag="cpy", name="cpy")
                # prologue + early-round chains copy on ACT (exp stream idle
                # or thin there); later rounds keep DVE
                if cpy_act or nt <= opts.get("qcopy_act_nt", 2):
                    nc.scalar.copy(cpy[:], pqk[:])
                else:
                    nc.vector.tensor_copy(cpy[:], pqk[:])
                t0 = ropep.tile([P, w], bf16, tag="t0", name="t0")
                sw = ropep.tile([P, w], bf16, tag="sw", name="sw")
                u0 = ropep.tile([P, w], bf16, tag="u0", name="u0")
                nc.vector.tensor_mul(t0[:], cpy[:], cossin[:, 0, nsl])
                nc.vector.stream_shuffle(sw[:], cpy[:], _PAIRSWAP)
                nc.vector.tensor_mul(u0[:], sw[:], cossin[:, 1, nsl])
                dst = qsb if mt < 2 else ksb
                nc.vector.tensor_add(dst[:, mt % 2, nsl], t0[:], u0[:])

            def v_chain(nt, tt, cpy_act=False):
                """V projection for one 128-token subtile (3-term fp8 DR).
                The psum holds 64·v; the copy descales by 1/64."""
                kt = nt * 4 + tt
                blk, off = kt // 2, (kt % 2) * P
                pv = ps_m.tile([P, HPC, DK], f32, tag="m", name="pv")
                ni = 0
                for jx, jw in ((0, 0), (1, 0), (0, 1)):
                    for t in range(KO // 2):
                        nc.tensor.matmul(
                            pv[:],
                            xt2[:, jx, blk, 2 * t:2 * t + 2, off:off + P],
                            wv2[:, jw, 2 * t:2 * t + 2, :],
                            start=(ni == 0),
                            stop=(ni == 3 * (KO // 2) - 1),
                            perf_mode=DR,
                        )
                        ni += 1
                if vcopy_eng == "dve" and not cpy_act:
                    nc.vector.tensor_scalar_mul(
                        vones[:, kt, :, 0:DK], pv[:], 1.0 / WSCALE
                    )
                else:
                    nc.scalar.mul(vones[:, kt, :, 0:DK], pv[:], 1.0 / WSCALE)

            def phase_b_fillers(nt):
                return [lambda mt=mt: qkv_chain(nt, mt) for mt in range(4)] + [
                    lambda tt=tt: v_chain(nt, tt) for tt in range(4)
                ]

            def oproj_fillers(qt, onorm_pack, pools=((ps_m, "m"),)):
                """Output projection chains for query tile qt (3-term fp8
                DoubleRow; psum lands at 64x so copies descale by 1/64).
                The later tiles run near the kernel tail where the exp
                stream has ended, so their copies alternate onto the idle
                Activation engine and their output DMAs are split to keep
                the last transfer small."""
                onorm, onorm8, rho8 = onorm_pack
                qsl = slice(qt * QTILE, (qt + 1) * QTILE)
                late = qt >= NQ - 2
                last = qt == NQ - 1
                ysb = ysbp.tile([P, KO, QTILE], bf16, tag="y", name="ysb")

                def ochain(ot):
                    pool, ptag = pools[ot % len(pools)]
                    py = pool.tile([P, QTILE], f32, tag=ptag, name="py")
                    otsl = slice(ot * P, (ot + 1) * P)
                    for c0 in (0, 256):
                        for ni, (j, mv) in enumerate(
                            ((0, onorm8), (0, rho8), (1, onorm8))
                        ):
                            nc.tensor.matmul(
                                py[:, c0:c0 + 256],
                                wo2[:, j, 0:2, otsl],
                                mv[:, 0:2, c0:c0 + 256],
                                start=(ni == 0),
                                stop=(ni == 2),
                                perf_mode=DR,
                            )
                    if "noy" in probe:
                        return
                    # Activation copies only where the exp stream has ended
                    # (the held-back tail chains); mid-round copies stay on
                    # DVE so they don't stretch the exp-bound final round
                    use_act = ycopy_eng == "act" or (
                        qt == NQ - 2 and ot >= opts.get("act_ot", 2)
                    )
                    if use_act:
                        nc.scalar.mul(ysb[:, ot, :], py[:], 1.0 / WSCALE)
                    else:
                        nc.vector.tensor_scalar_mul(
                            ysb[:, ot, :], py[:], 1.0 / WSCALE
                        )
                    # split the late tiles' output DMAs so the tail after the
                    # last matmul is a small transfer, not a whole tile; the
                    # qt==NQ-2 tile ships in fine grain because its last
                    # chunks land inside the final tile's drain window
                    if late and not last:
                        if opts.get("late_dma", "coarse") == "fine":
                            if ot in (1, 3, 5, 7):
                                nc.sync.dma_start(
                                    out_d[:, ot - 1:ot + 1, qsl],
                                    ysb[:, ot - 1:ot + 1, :],
                                )
                        else:
                            if ot == 3:
                                nc.sync.dma_start(out_d[:, 0:4, qsl], ysb[:, 0:4, :])
                            elif ot == KO - 1:
                                nc.sync.dma_start(out_d[:, 4:8, qsl], ysb[:, 4:8, :])
                    elif last and ot == 5:
                        nc.sync.dma_start(out_d[:, 4:6, qsl], ysb[:, 4:6, :])
                    elif last and ot == 6:
                        nc.sync.dma_start(out_d[:, 6:7, qsl], ysb[:, 6:7, :])
                    elif last and ot == KO - 1:
                        nc.sync.dma_start(out_d[:, 7:8, qsl], ysb[:, 7:8, :])
                    elif not last and ot == KO - 1:
                        nc.sync.dma_start(out_d[:, :, qsl], ysb[:])

                return [lambda ot=ot: ochain(ot) for ot in range(KO)]

            def run_round(qt, fillers, pre=(), defer_at=4, keep_tail=2,
                          final=False):
                """Attention for query tile qt, interleaved with fillers.

                ``pre`` holds deferred closures (the previous round's softmax
                normalization chains): they are emitted a few slots in, so
                they don't head-of-line-block the DVE queue ahead of this
                round's first diagonal mask multiplies. This round's own norm
                chains are returned as closures for the next round (the last
                round emits them inline). Filler pacing is weighted by each
                slot's PE deficit: a diagonal slot has little matmul work but
                a full exp, so it gets more filler coverage.
                """
                onorm = onormp.tile([P, 2, QTILE], bf16, tag="on", name="onorm")
                if qt < NQ - 1:
                    # fp8 quantized onorm + residual for the fp8 output
                    # projection (produced on the otherwise-idle Pool engine)
                    onorm8 = on8p.tile([P, 2, QTILE], fp8, tag="o8", name="on8")
                    rho8 = on8p.tile([P, 2, QTILE], fp8, tag="r8", name="rho8")
                else:
                    onorm8 = rho8 = None
                nkt = 4 * (qt + 1)
                nfill = len(fillers)
                emitted = [0]
                # deferred closures keyed by the slot index that releases them
                deferq = {defer_at: list(pre)}

                # per-slot PE deficit weight: full-slot ACT time is roughly
                # constant, PE slot work scales with the causal width
                weights = []
                for hp in range(2):
                    for kt in (range(4 * qt, nkt) if diag_first else range(nkt)):
                        r = kt - 4 * qt
                        rq = max(r, 0) * P
                        weights.append(1.0 + 3.0 * rq / QTILE)
                    if diag_first:
                        weights.extend([1.0] * (4 * qt))
                total_w = sum(weights)
                nslots = len(weights)
                cum = 0.0

                front = 1.5 if final else 1.0
                # fillers may consume onorm written by the deferred norm
                # closures in ``pre`` — hold them until those are emitted
                fill_gate = defer_at if pre else 0

                def pace(i):
                    nonlocal cum
                    cum += weights[i]
                    if i < fill_gate:
                        return
                    want = min(
                        nfill - keep_tail,
                        int(front * nfill * cum / total_w + 1e-9),
                    )
                    while emitted[0] < want:
                        fillers[emitted[0]]()
                        emitted[0] += 1

                slot_idx = 0
                for hp in range(2):
                    po = [
                        ps_o.tile([P, QTILE], f32, tag="o", name=f"po{i}")
                        for i in range(2)
                    ]
                    if diag_first:
                        kt_order = list(range(4 * qt, nkt)) + list(range(4 * qt))
                    else:
                        kt_order = list(range(nkt))
                    pending = []

                    def emit_pv(idx, prab, rq, po=po, hp=hp, nkt=nkt):
                        if "nopv" in probe:
                            if idx == 0:
                                for half in range(2):
                                    nc.vector.memset(po[half][: DK + 1, 0:1], 1.0)
                            return
                        for half in range(2):
                            h = 2 * hp + half
                            nc.tensor.matmul(
                                po[half][: DK + 1, rq:],
                                vones[:, kt_order[idx], h, :],
                                prab[:, half, rq:],
                                start=(idx == 0),
                                stop=(idx == nkt - 1),
                            )

                    for idx, kt in enumerate(kt_order):
                        ktsl = slice(kt * P, (kt + 1) * P)
                        r = kt - 4 * qt
                        rq = max(r, 0) * P  # causally-valid q range start
                        pstile = ps_s.tile([P, 2, QTILE], f32, tag="s", name="ps")
                        if "noscores" in probe:
                            nc.vector.memset(pstile[:, :, rq:rq + 1], 0.0)
                        if "noscores" not in probe:
                            for half in range(2):
                                psl = slice(half * 64, (half + 1) * 64)
                                nc.tensor.matmul(
                                    pstile[:, half, rq:],
                                    ksb[psl, hp, ktsl],
                                    qsb[psl, hp, qt * QTILE + rq:(qt + 1) * QTILE],
                                    start=True,
                                    stop=True,
                                    tile_position=(half * 64, 0),
                                )
                        prab = probsp.tile([P, 2, QTILE], bf16, tag="pr", name="pr")
                        if "noexp" not in probe:
                            nc.scalar.activation(
                                prab[:, :, rq:], pstile[:, :, rq:], Exp, scale=0.125
                            )
                        else:
                            nc.vector.memset(prab[:, :, rq:rq + 1], 1.0)
                        if r >= 0:
                            # mask only the diagonal 128-wide band
                            nc.vector.tensor_mul(
                                prab[:, :, rq:rq + P],
                                prab[:, :, rq:rq + P],
                                maskt[:, None, :].to_broadcast([P, 2, P]),
                            )
                        pending.append((idx, prab, rq))
                        if len(pending) > min(pvdepth, nkt - 2):
                            emit_pv(*pending.pop(0))
                        pace(slot_idx)
                        slot_idx += 1
                        for cl in deferq.pop(slot_idx, ()):
                            cl()
                    for args in pending:
                        emit_pv(*args)
                        # a filler between flushed PVs: each PV may wait on
                        # its exp, so give the PE other work in between
                        if emitted[0] < nfill - keep_tail:
                            fillers[emitted[0]]()
                            emitted[0] += 1

                    def norm_chain(hp=hp, po=po):
                        for half in range(2):
                            rc = yp.tile([1, QTILE], f32, tag="rc", name="rc")
                            nc.vector.reciprocal(rc[:], po[half][DK:DK + 1, :])
                            rb = yp.tile([DK, QTILE], f32, tag="rb", name="rb")
                            nc.gpsimd.partition_broadcast(
                                rb[:], rc[:], channels=DK
                            )
                            nc.vector.tensor_mul(
                                onorm[64 * half:64 * half + 64, hp, :],
                                po[half][0:DK, :],
                                rb[:],
                            )
                        if onorm8 is not None:
                            nc.gpsimd.tensor_copy(
                                onorm8[:, hp, :], onorm[:, hp, :]
                            )
                            nc.gpsimd.tensor_sub(
                                rho8[:, hp, :], onorm[:, hp, :], onorm8[:, hp, :]
                            )

                    if hp == 0:
                        # emit a few slots into hp1 so it doesn't block hp1's
                        # diagonal mask multiplies in the DVE queue
                        deferq.setdefault(nkt + defer_at, []).append(norm_chain)
                    else:
                        deferred = norm_chain

                for cls in deferq.values():  # anything not yet released
                    for cl in cls:
                        cl()
                if final:
                    # norm chain ahead of the held-back fillers' DVE copies,
                    # which then give the PE work during its DVE/Pool latency
                    deferred()
                    deferred = None
                while emitted[0] < nfill:
                    fillers[emitted[0]]()
                    emitted[0] += 1
                return (onorm, onorm8, rho8), deferred

            # software pipeline: round t runs attention(t) interleaved with
            # filler matmul chains. The projections of tile t+1 fill round t;
            # output projections are pushed two rounds late (oproj(t) fills
            # round t+2) because the last round is exp-bound on the
            # Activation engine and needs all the spare PE work it can get.
            def body():
                # tile-0 projections, split in 256-token halves so the first
                # chain starts as soon as the first xt half lands; the second
                # warmup burst bridges until the mt1-3 weights and second xt
                # half arrive (hp0's rope deps, mt 0 and 2, come first so
                # round 0's scores unblock early)
                n_warm2 = opts.get("n_warm2", 6)
                qkv_chain(0, 0, 0, 256, cpy_act=True)
                if n_warm2:
                    wps2 = ps_s.tile([P, 256], f32, tag="s", name="warm2")
                    for i in range(n_warm2):
                        nc.tensor.matmul(
                            wps2[:],
                            scratch[:, 0:P],
                            scratch[:],
                            start=(i == 0),
                            stop=(i == n_warm2 - 1),
                        )
                for mt in (2, 1, 3):
                    qkv_chain(0, mt, 0, 256, cpy_act=True)
                for mt in (0, 2, 1, 3):
                    qkv_chain(0, mt, 256, QTILE, cpy_act=True)
                for tt in range(4):
                    v_chain(0, tt, cpy_act=True)
                onorms = {}
                pre = ()
                # which earlier tiles' output projections fill each round
                oproj_sched = opts.get("oproj_sched", {2: (0, 1), 3: (2,)})
                for t in range(NQ):
                    fillers = []
                    if t + 1 < NQ:
                        fillers += phase_b_fillers(t + 1)
                    for qo in oproj_sched.get(t, ()):
                        # the last round's held-back tail chains rotate
                        # through both free psum pools so they aren't
                        # copy-paced through a single 2-slot ring
                        pl = (
                            ((ps_m, "m"), (ps_m, "m"), (ps_s, "s"), (ps_m, "m"),
                             (ps_s, "s"), (ps_m, "m"), (ps_s, "s"), (ps_m, "m"))
                            if t == NQ - 1
                            else ((ps_m, "m"),)
                        )
                        fillers += oproj_fillers(qo, onorms.pop(qo), pools=pl)
                    onorms[t], deferred = run_round(
                        t, fillers, pre=pre,
                        keep_tail=opts.get("keep_tail", 8) if t == NQ - 1 else 2,
                        final=t == NQ - 1,
                    )
                    pre = (deferred,) if deferred is not None else ()
                # final output projection: the kj=0 halves only need hp0's
                # normalized output (ready mid-round), so they run during the
                # hp1 norm chain's DVE/Pool latency; kj=1 + copies follow
                qt = NQ - 1
                onorm = onorms[qt][0]
                qsl = slice(qt * QTILE, (qt + 1) * QTILE)
                ysb = ysbp.tile([P, KO, QTILE], bf16, tag="y", name="ysb")
                pools4 = [(ps_m, "m"), (ps_s, "s")] * 2
                pys = []
                for ot in range(4):
                    pool, ptag = pools4[ot]
                    py = pool.tile([P, QTILE], f32, tag=ptag, name="py")
                    nc.tensor.matmul(
                        py[:], wo[:, 0, ot * P:(ot + 1) * P], onorm[:, 0, :],
                        start=True, stop=False,
                    )
                    pys.append(py)

                def fin_copy(ot):
                    if ot % 2 == 1:
                        nc.scalar.copy(ysb[:, ot, :], pys[ot][:])
                    else:
                        nc.vector.tensor_copy(ysb[:, ot, :], pys[ot][:])

                for ot in range(4):
                    nc.tensor.matmul(
                        pys[ot][:], wo[:, 1, ot * P:(ot + 1) * P],
                        onorm[:, 1, :], start=False, stop=True,
                    )
                    fin_copy(ot)
                    if ot % 2 == 1:  # ship every pair as soon as it's staged
                        nc.sync.dma_start(
                            out_d[:, ot - 1:ot + 1, qsl], ysb[:, ot - 1:ot + 1, :]
                        )
                for ot in range(4, KO):
                    pool, ptag = pools4[ot - 4]
                    py = pool.tile([P, QTILE], f32, tag=ptag, name="py")
                    for kj in range(2):
                        nc.tensor.matmul(
                            py[:], wo[:, kj, ot * P:(ot + 1) * P],
                            onorm[:, kj, :], start=(kj == 0), stop=(kj == 1),
                        )
                    pys.append(py)
                    fin_copy(ot)
                    if opts.get("fin_dma", "pairs") == "pairs":
                        if ot % 2 == 1:
                            nc.sync.dma_start(
                                out_d[:, ot - 1:ot + 1, qsl], ysb[:, ot - 1:ot + 1, :]
                            )
                    else:
                        if ot == 5:
                            nc.sync.dma_start(out_d[:, 4:6, qsl], ysb[:, 4:6, :])
                        elif ot == 6:
                            nc.sync.dma_start(out_d[:, 6:7, qsl], ysb[:, 6:7, :])
                        elif ot == KO - 1:
                            nc.sync.dma_start(out_d[:, 7:8, qsl], ysb[:, 7:8, :])

            if loop:
                with tc.For_i(0, reps, 1):
                    body()
            else:
                for _rep in range(reps):
                    body()
    nc.compile()
    return nc


def _feature_major(rows_x_d, dt=BF):
    """[M, D] (row-major, d = ko*128+ki) -> [P, KO, M] in dtype dt."""
    m = rows_x_d.shape[0]
    return np.ascontiguousarray(
        rows_x_d.T.reshape(KO, P, m).transpose(1, 0, 2)
    ).astype(dt)


def _f8_pair(a):
    """Quantize float32 array to (fp8, fp8 residual)."""
    a8 = a.astype(F8)
    r8 = (a - a8.astype(np.float32)).astype(F8)
    return a8, r8


def _prep_in_maps(x, W_qkv, W_o, token_positions):
    x = np.asarray(x, dtype=np.float32)
    W_qkv = np.asarray(W_qkv, dtype=np.float32)
    W_o = np.asarray(W_o, dtype=np.float32)
    pos = np.asarray(token_positions)

    inv_freq = 1.0 / (
        np.float32(THETA) ** (np.arange(0, DK, 2, dtype=np.float32) / np.float32(DK))
    )
    inv_freq = inv_freq.astype(np.float32)
    freqs = pos.astype(np.float32)[:, :, None] * inv_freq[None, None, :]  # [B,S,32]
    cos = np.cos(freqs).astype(np.float32)
    sin = np.sin(freqs).astype(np.float32)

    jidx = (np.arange(P) % DK) // 2
    sign = np.where(np.arange(P) % 2 == 0, -1.0, 1.0).astype(np.float32)
    # cos/sin tables carry the 1/WSCALE descale of the fp8 QKV psum;
    # combined [P, 2, S] (dim1 = cos, sin) for single-DMA loads
    cs_tab = []
    for b in range(B):
        c = np.ascontiguousarray(cos[b].T[jidx] / WSCALE).astype(BF)
        s = np.ascontiguousarray(sin[b].T[jidx] * sign[:, None] / WSCALE).astype(BF)
        cs_tab.append(np.ascontiguousarray(np.stack([c, s], axis=1)))

    masks = (np.arange(P)[:, None] <= np.arange(P)[None, :]).astype(BF)  # tril^T

    xT2 = []
    for b in range(B):
        fm = np.ascontiguousarray(
            x[b].T.reshape(KO, P, S).transpose(1, 0, 2)
        ).astype(np.float32)
        a8, r8 = _f8_pair(fm)
        # [P, 2, KO, S] -> blocked [P, 2, S//256, KO, 256]
        st = np.stack([a8, r8], axis=1).reshape(P, 2, KO, S // 256, 256)
        xT2.append(np.ascontiguousarray(st.transpose(0, 1, 3, 2, 4)))

    in_maps = []
    for c in range(N_CORES):
        b, hg = divmod(c, 4)
        heads = range(hg * HPC, (hg + 1) * HPC)
        q_rows = np.concatenate([W_qkv[h * DK:(h + 1) * DK] for h in heads])
        k_rows = np.concatenate(
            [W_qkv[D + h * DK:D + (h + 1) * DK] for h in heads]
        )
        v_rows = np.concatenate(
            [W_qkv[2 * D + h * DK:2 * D + (h + 1) * DK] for h in heads]
        )
        wqk_fm = _feature_major(
            np.concatenate([q_rows, k_rows]) * WSCALE, np.float32
        )  # [P, KO, 512]
        # regroup as [P, 4(mt), KO, 128] so each mt slice is one contiguous
        # DMA (the 128-col stationary tiles of the QKV matmul)
        wqk_f = np.ascontiguousarray(
            wqk_fm.reshape(P, KO, 4, P).transpose(0, 2, 1, 3)
        )
        wqk8, wqks8 = _f8_pair(wqk_f)
        wv8, wvs8 = _f8_pair(_feature_major(v_rows * WSCALE, np.float32))
        wo_sub = W_o[:, hg * 256:(hg + 1) * 256]  # [D, 256]
        wo_fm = np.ascontiguousarray(
            wo_sub.T.reshape(2, P, D).transpose(1, 0, 2)
        ).astype(np.float32)  # [P, 2, D]
        # onorm arrives at 8x (ones row = 1/8): bf16 final-tile weights are
        # pre-divided by 8; fp8 weights are pre-multiplied by 8 (net 64x psum)
        wo8, wos8 = _f8_pair(wo_fm * 8.0)
        in_maps.append(
            {
                "xT2": xT2[b],
                "wqk2": np.ascontiguousarray(np.stack([wqk8, wqks8], axis=1)),
                "wv2": np.ascontiguousarray(np.stack([wv8, wvs8], axis=1)),
                "wo": (wo_fm / 8.0).astype(BF),
                "wo2": np.ascontiguousarray(np.stack([wo8, wos8], axis=1)),
                "cossin": cs_tab[b],
                "masks": masks,
            }
        )
    return in_maps


def _get_nc(reps=1, loop=False, probe=(), opts=None):
    key = f"nc{reps}_{loop}_{sorted(probe)}_{sorted((opts or {}).items())}"
    if key not in _CACHE:
        _CACHE[key] = _build_nc(reps, loop, probe, opts)
    return _CACHE[key]


def kernel(x, W_qkv, W_o, token_positions):
    nc = _get_nc()
    in_maps = _prep_in_maps(x, W_qkv, W_o, token_positions)
    res = run_bass_kernel_spmd(nc, in_maps, core_ids=list(range(N_CORES)))
    out = np.zeros((B, S, D), dtype=np.float32)
    for c in range(N_CORES):
        b = c // 4
        # out_t is [P, 8(ot), S] bf16: row d = ot*128 + p of y^T
        yt = np.asarray(res.results[c]["out_t"], dtype=np.float32)
        out[b] += yt.transpose(1, 0, 2).reshape(D, S).T
    return out



# revision 58
# speedup vs baseline: 1.0338x; 1.0039x over previous
"""Causal multi-head self-attention with RoPE on 8 TRN2 NeuronCores.

Sharding: core c handles batch b = c // 4 and heads [4*(c%4), 4*(c%4)+4).
All cores run one SPMD Bass program; per-core behavior comes entirely from
the data (pre-sliced weights, per-batch activations). Each core computes its
4 heads' attention and the partial output projection y^T = W_o_slice^T @ out;
the host sums the 4 partials per batch (the "all-reduce" of the TP split).

Device layout is feature-major throughout: x^T [D, S] feeds QKV as the
moving operand; scores are computed transposed (k on partitions, q free) so
the softmax denominator falls out of a ones-row appended to V in the PV
matmul, and the attention output emerges as out^T [d, q], which is exactly
the moving operand the output projection needs. RoPE is applied on the QKV
PSUM with a pair-swap stream shuffle + host-precomputed cos/sin tables.

The emission is software-pipelined: attention slots for query tile t (whose
per-slot rate is bounded by the Activation engine's exp) are interleaved
with the QKV/V projection chains of tile t+1 and the output projection of
tile t-1, so the Tensor engine always has filler matmuls while exp catches
up. Compute dtype bf16 (fp32 accumulate), f32 in, bf16 partials out (host
accumulates the 4 per-batch partials in f32).

The QKV projection runs in fp8(e4m3) DoubleRow perf mode (0.5 PE cycles per
output row, two 128-deep contraction subtiles per instruction) using a
3-term residual expansion x·W ~= x8·w8 + r8·w8 + x8·s8 with r8 = fp8(x-x8),
s8 = fp8(64W - w8); W is pre-scaled by 64 so its residual clears the e4m3
subnormal floor. All quantization happens host-side; the 1/64 descale is
folded into the cos/sin tables for Q/K and into the V psum-copy multiplier.
"""

import sys

sys.path.insert(0, "/opt/trn_rl_repo")

import numpy as np
import ml_dtypes

import concourse.bass as bass
import concourse.bacc as bacc
import concourse.mybir as mybir
import concourse.tile as tile
from concourse.bass_utils import run_bass_kernel_spmd

B, S, D = 2, 2048, 1024
H, DK = 16, 64
THETA = 10000.0
HPC = 4  # heads per core
P = 128
KO = D // P  # 8 contraction subtiles for the projections
QTILE = 512
NQ = S // QTILE  # 4 query tiles
NKT = S // P  # 16 key-token tiles
N_CORES = 8
BF = ml_dtypes.bfloat16

_PAIRSWAP = [i + 1 if i % 2 == 0 else i - 1 for i in range(32)]

F8 = ml_dtypes.float8_e4m3
WSCALE = 64.0  # host pre-scale on W_qkv so fp8 residuals stay out of subnormals

_CACHE = {}


def _build_nc(reps=1, loop=False, probe=(), opts=None):
    probe = set(probe)
    opts = dict(opts or {})
    vcopy_eng = opts.get("vcopy", "act")
    ycopy_eng = opts.get("ycopy", "dve")
    pvdepth = opts.get("pvdepth", 8)
    probs_bufs = opts.get("probs_bufs", 10)
    rope_bufs = opts.get("rope_bufs", 8)
    yp_bufs = opts.get("yp_bufs", 8)
    on_bufs = opts.get("on_bufs", 3)
    diag_first = opts.get("diag_first", False)
    n_warm = opts.get("n_warm", 12)
    f32 = mybir.dt.float32
    bf16 = mybir.dt.bfloat16
    fp8 = mybir.dt.float8e4
    Exp = mybir.ActivationFunctionType.Exp
    DR = mybir.MatmulPerfMode.DoubleRow

    nc = bacc.Bacc()
    # paired-value/residual tensors are combined on dim1 so each slice is a
    # single DMA (HWDGE generation is a serial 625ns/DMA device); x is blocked
    # by 256-token chunks so every DMA slice is a 2KB contiguous run (descs
    # under 512B pay a 2x DMA latency multiplier)
    NB = S // 256
    xT_d = nc.dram_tensor("xT2", [P, 2, NB, KO, 256], fp8, kind="ExternalInput")
    wqk_d = nc.dram_tensor("wqk2", [P, 2, 4, KO, P], fp8, kind="ExternalInput")
    wv_d = nc.dram_tensor("wv2", [P, 2, KO, HPC * DK], fp8, kind="ExternalInput")
    wo_d = nc.dram_tensor("wo", [P, 2, D], bf16, kind="ExternalInput")
    wo2_d = nc.dram_tensor("wo2", [P, 2, 2, D], fp8, kind="ExternalInput")
    cs_d = nc.dram_tensor("cossin", [P, 2, S], bf16, kind="ExternalInput")
    mask_d = nc.dram_tensor("masks", [P, P], bf16, kind="ExternalInput")
    out_d = nc.dram_tensor("out_t", [P, KO, S], bf16, kind="ExternalOutput")

    with tile.TileContext(nc) as tc:
        with (
            tc.tile_pool(name="const", bufs=1) as cp,
            tc.tile_pool(name="rope", bufs=rope_bufs) as ropep,
            tc.tile_pool(name="probs", bufs=probs_bufs) as probsp,
            tc.tile_pool(name="onp", bufs=on_bufs) as onormp,
            tc.tile_pool(name="on8", bufs=on_bufs) as on8p,
            tc.tile_pool(name="yp", bufs=yp_bufs) as yp,
            tc.tile_pool(name="ysb", bufs=2) as ysbp,
            tc.tile_pool(name="ps_s", bufs=2, space="PSUM") as ps_s,
            tc.tile_pool(name="ps_o", bufs=2, space="PSUM") as ps_o,
            tc.tile_pool(name="ps_m", bufs=2, space="PSUM") as ps_m,
        ):
            xt2 = cp.tile([P, 2, NB, KO, 256], fp8, tag="xT2")
            wqk2 = cp.tile([P, 2, 4, KO, P], fp8, tag="wqk2")
            wv2 = cp.tile([P, 2, KO, HPC * DK], fp8, tag="wv2")
            wo = cp.tile([P, 2, D], bf16, tag="wo")
            wo2 = cp.tile([P, 2, 2, D], fp8, tag="wo2")
            cossin = cp.tile([P, 2, S], bf16, tag="cossin")
            maskt = cp.tile([P, P], bf16, tag="mask")
            vones = cp.tile([P, NKT, HPC, DK + 1], bf16, tag="vones")
            qsb = cp.tile([P, 2, S], bf16, tag="qsb")
            ksb = cp.tile([P, 2, S], bf16, tag="ksb")
            # raw (non-tile) sbuf tensor: dependency-free so the warmup
            # matmuls can start immediately without waiting on a memset
            scratch = nc.alloc_sbuf_tensor("warm_scr", [P, 256], bf16)

            # PE pre-ramp: the p-state model runs matmuls at reduced clock
            # until the engine has been continuously busy ~3us. Warm it up on
            # scratch data while the first input DMAs are in flight; the ramp
            # then carries into the real matmul stream with no idle gap.
            # the ones row is 1/8 so onorm comes out as 8*att, matching the
            # x8 pre-scale of the fp8 output-projection weights.
            # scratch is intentionally never initialized: the warmup matmuls
            # only exist to hold the PE p-state ramp, their psum is never
            # read, and skipping the memset lets the PE start immediately.
            nc.vector.memset(vones[:, :, :, DK:DK + 1], 0.125)
            if n_warm:
                wps = ps_s.tile([P, 256], f32, tag="s", name="warm")
                for i in range(n_warm):
                    nc.tensor.matmul(
                        wps[:],
                        scratch[:, 0:P],
                        scratch[:],
                        start=(i == 0),
                        stop=(i == n_warm - 1),
                    )

            # input DMAs, all on the SP queue. Each DMA costs one 625ns HWDGE
            # slot and its transfer serializes on the single DMA device, so
            # order = need-time: the first QKV chain's operands in fine grain
            # (the chain starts as soon as ko 0 lands and is paced by the
            # rest), then everything else batched coarse.
            # prologue slices in consumption order: chain A (x8*w8) of mt 0
            # starts after DMAs 1-2; the r8/s8 terms and the other mt blocks
            # follow; cos/sin splits so the first rope isn't behind xt blk1
            nc.sync.dma_start(wqk2[:, :, 0], wqk_d[:, :, 0])
            nc.sync.dma_start(xt2[:, 0, 0], xT_d[:, 0, 0])
            nc.sync.dma_start(xt2[:, 1, 0], xT_d[:, 1, 0])
            nc.sync.dma_start(wqk2[:, :, 2], wqk_d[:, :, 2])
            nc.sync.dma_start(cossin[:, :, 0:256], cs_d[:, :, 0:256])
            nc.sync.dma_start(wqk2[:, :, 1], wqk_d[:, :, 1])
            nc.sync.dma_start(wqk2[:, :, 3], wqk_d[:, :, 3])
            nc.sync.dma_start(xt2[:, 0, 1], xT_d[:, 0, 1])
            nc.sync.dma_start(xt2[:, 1, 1], xT_d[:, 1, 1])
            nc.sync.dma_start(cossin[:, :, 256:QTILE], cs_d[:, :, 256:QTILE])
            nc.sync.dma_start(maskt[:], mask_d[:])
            nc.sync.dma_start(wv2[:], wv_d[:])
            for nt in range(1, NQ):
                sl = slice(nt * QTILE, (nt + 1) * QTILE)
                nc.sync.dma_start(xt2[:, :, 2 * nt:2 * nt + 2], xT_d[:, :, 2 * nt:2 * nt + 2])
                nc.sync.dma_start(cossin[:, :, sl], cs_d[:, :, sl])
                if nt == 2:
                    nc.sync.dma_start(wo2[:], wo2_d[:])
                if nt == 3:
                    nc.sync.dma_start(wo[:], wo_d[:])

            def qkv_chain(nt, mt, lo=0, hi=QTILE, cpy_act=False):
                """One 128-col QKV chain + psum copy + rope to qsb/ksb.

                3-term fp8 DoubleRow: x8·w8 + r8·w8 + x8·s8 in 256-col
                chunks (DoubleRow moving free = 2N <= 512)."""
                nsl = slice(nt * QTILE + lo, nt * QTILE + hi)
                w = hi - lo
                pqk = ps_m.tile([P, w], f32, tag="m", name="pqk")
                for c0 in range(0, w, 256):
                    cw = min(256, w - c0)
                    blk = (nt * QTILE + lo + c0) // 256
                    ni = 0
                    for jw, jx in ((0, 0), (0, 1), (1, 0)):
                        for t in range(KO // 2):
                            nc.tensor.matmul(
                                pqk[:, c0:c0 + cw],
                                wqk2[:, jw, mt, 2 * t:2 * t + 2, :],
                                xt2[:, jx, blk, 2 * t:2 * t + 2, 0:cw],
                                start=(ni == 0),
                                stop=(ni == 3 * (KO // 2) - 1),
                                perf_mode=DR,
                            )
                            ni += 1
                cpy = ropep.tile([P, w], bf16, tag="cpy", name="cpy")
                # prologue + early-round chains copy on ACT (exp stream idle
                # or thin there); later rounds keep DVE
                if cpy_act or nt <= opts.get("qcopy_act_nt", 2):
                    nc.scalar.copy(cpy[:], pqk[:])
                else:
                    nc.vector.tensor_copy(cpy[:], pqk[:])
                t0 = ropep.tile([P, w], bf16, tag="t0", name="t0")
                sw = ropep.tile([P, w], bf16, tag="sw", name="sw")
                u0 = ropep.tile([P, w], bf16, tag="u0", name="u0")
                nc.vector.tensor_mul(t0[:], cpy[:], cossin[:, 0, nsl])
                nc.vector.stream_shuffle(sw[:], cpy[:], _PAIRSWAP)
                nc.vector.tensor_mul(u0[:], sw[:], cossin[:, 1, nsl])
                dst = qsb if mt < 2 else ksb
                nc.vector.tensor_add(dst[:, mt % 2, nsl], t0[:], u0[:])

            def v_chain(nt, tt, cpy_act=False):
                """V projection for one 128-token subtile (3-term fp8 DR).
                The psum holds 64·v; the copy descales by 1/64."""
                kt = nt * 4 + tt
                blk, off = kt // 2, (kt % 2) * P
                pv = ps_m.tile([P, HPC, DK], f32, tag="m", name="pv")
                ni = 0
                for jx, jw in ((0, 0), (1, 0), (0, 1)):
                    for t in range(KO // 2):
                        nc.tensor.matmul(
                            pv[:],
                            xt2[:, jx, blk, 2 * t:2 * t + 2, off:off + P],
                            wv2[:, jw, 2 * t:2 * t + 2, :],
                            start=(ni == 0),
                            stop=(ni == 3 * (KO // 2) - 1),
                            perf_mode=DR,
                        )
                        ni += 1
                if vcopy_eng == "dve" and not cpy_act:
                    nc.vector.tensor_scalar_mul(
                        vones[:, kt, :, 0:DK], pv[:], 1.0 / WSCALE
                    )
                else:
                    nc.scalar.mul(vones[:, kt, :, 0:DK], pv[:], 1.0 / WSCALE)

            def phase_b_fillers(nt):
                return [lambda mt=mt: qkv_chain(nt, mt) for mt in range(4)] + [
                    lambda tt=tt: v_chain(nt, tt) for tt in range(4)
                ]

            def oproj_fillers(qt, onorm_pack, pools=((ps_m, "m"),)):
                """Output projection chains for query tile qt (3-term fp8
                DoubleRow; psum lands at 64x so copies descale by 1/64).
                The later tiles run near the kernel tail where the exp
                stream has ended, so their copies alternate onto the idle
                Activation engine and their output DMAs are split to keep
                the last transfer small."""
                onorm, onorm8, rho8 = onorm_pack
                qsl = slice(qt * QTILE, (qt + 1) * QTILE)
                late = qt >= NQ - 2
                last = qt == NQ - 1
                ysb = ysbp.tile([P, KO, QTILE], bf16, tag="y", name="ysb")

                def ochain(ot):
                    pool, ptag = pools[ot % len(pools)]
                    py = pool.tile([P, QTILE], f32, tag=ptag, name="py")
                    otsl = slice(ot * P, (ot + 1) * P)
                    for c0 in (0, 256):
                        for ni, (j, mv) in enumerate(
                            ((0, onorm8), (0, rho8), (1, onorm8))
                        ):
                            nc.tensor.matmul(
                                py[:, c0:c0 + 256],
                                wo2[:, j, 0:2, otsl],
                                mv[:, 0:2, c0:c0 + 256],
                                start=(ni == 0),
                                stop=(ni == 2),
                                perf_mode=DR,
                            )
                    if "noy" in probe:
                        return
                    # Activation copies only where the exp stream has ended
                    # (the held-back tail chains); mid-round copies stay on
                    # DVE so they don't stretch the exp-bound final round
                    use_act = ycopy_eng == "act" or (
                        qt == NQ - 2 and ot >= opts.get("act_ot", 2)
                    )
                    if use_act:
                        nc.scalar.mul(ysb[:, ot, :], py[:], 1.0 / WSCALE)
                    else:
                        nc.vector.tensor_scalar_mul(
                            ysb[:, ot, :], py[:], 1.0 / WSCALE
                        )
                    # split the late tiles' output DMAs so the tail after the
                    # last matmul is a small transfer, not a whole tile; the
                    # qt==NQ-2 tile ships in fine grain because its last
                    # chunks land inside the final tile's drain window
                    if late and not last:
                        if opts.get("late_dma", "coarse") == "fine":
                            if ot in (1, 3, 5, 7):
                                nc.sync.dma_start(
                                    out_d[:, ot - 1:ot + 1, qsl],
                                    ysb[:, ot - 1:ot + 1, :],
                                )
                        else:
                            if ot == 3:
                                nc.sync.dma_start(out_d[:, 0:4, qsl], ysb[:, 0:4, :])
                            elif ot == KO - 1:
                                nc.sync.dma_start(out_d[:, 4:8, qsl], ysb[:, 4:8, :])
                    elif last and ot == 5:
                        nc.sync.dma_start(out_d[:, 4:6, qsl], ysb[:, 4:6, :])
                    elif last and ot == 6:
                        nc.sync.dma_start(out_d[:, 6:7, qsl], ysb[:, 6:7, :])
                    elif last and ot == KO - 1:
                        nc.sync.dma_start(out_d[:, 7:8, qsl], ysb[:, 7:8, :])
                    elif not last and ot == KO - 1:
                        nc.sync.dma_start(out_d[:, :, qsl], ysb[:])

                return [lambda ot=ot: ochain(ot) for ot in range(KO)]

            def run_round(qt, fillers, pre=(), defer_at=None, keep_tail=2,
                          final=False):
                if defer_at is None:
                    defer_at = opts.get("defer_at", 3)
                """Attention for query tile qt, interleaved with fillers.

                ``pre`` holds deferred closures (the previous round's softmax
                normalization chains): they are emitted a few slots in, so
                they don't head-of-line-block the DVE queue ahead of this
                round's first diagonal mask multiplies. This round's own norm
                chains are returned as closures for the next round (the last
                round emits them inline). Filler pacing is weighted by each
                slot's PE deficit: a diagonal slot has little matmul work but
                a full exp, so it gets more filler coverage.
                """
                onorm = onormp.tile([P, 2, QTILE], bf16, tag="on", name="onorm")
                if qt < NQ - 1:
                    # fp8 quantized onorm + residual for the fp8 output
                    # projection (produced on the otherwise-idle Pool engine)
                    onorm8 = on8p.tile([P, 2, QTILE], fp8, tag="o8", name="on8")
                    rho8 = on8p.tile([P, 2, QTILE], fp8, tag="r8", name="rho8")
                else:
                    onorm8 = rho8 = None
                nkt = 4 * (qt + 1)
                nfill = len(fillers)
                emitted = [0]
                # deferred closures keyed by the slot index that releases them
                deferq = {defer_at: list(pre)}

                # per-slot PE deficit weight: full-slot ACT time is roughly
                # constant, PE slot work scales with the causal width
                weights = []
                for hp in range(2):
                    for kt in (range(4 * qt, nkt) if diag_first else range(nkt)):
                        r = kt - 4 * qt
                        rq = max(r, 0) * P
                        weights.append(1.0 + 3.0 * rq / QTILE)
                    if diag_first:
                        weights.extend([1.0] * (4 * qt))
                total_w = sum(weights)
                nslots = len(weights)
                cum = 0.0

                front = opts.get("front_final", 1.5) if final else opts.get("front", 1.0)
                # fillers may consume onorm written by the deferred norm
                # closures in ``pre`` — hold them until those are emitted
                fill_gate = defer_at if pre else 0

                def pace(i):
                    nonlocal cum
                    cum += weights[i]
                    if i < fill_gate:
                        return
                    want = min(
                        nfill - keep_tail,
                        int(front * nfill * cum / total_w + 1e-9),
                    )
                    while emitted[0] < want:
                        fillers[emitted[0]]()
                        emitted[0] += 1

                slot_idx = 0
                for hp in range(2):
                    po = [
                        ps_o.tile([P, QTILE], f32, tag="o", name=f"po{i}")
                        for i in range(2)
                    ]
                    if diag_first:
                        kt_order = list(range(4 * qt, nkt)) + list(range(4 * qt))
                    else:
                        kt_order = list(range(nkt))
                    pending = []

                    def emit_pv(idx, prab, rq, po=po, hp=hp, nkt=nkt):
                        if "nopv" in probe:
                            if idx == 0:
                                for half in range(2):
                                    nc.vector.memset(po[half][: DK + 1, 0:1], 1.0)
                            return
                        for half in range(2):
                            h = 2 * hp + half
                            nc.tensor.matmul(
                                po[half][: DK + 1, rq:],
                                vones[:, kt_order[idx], h, :],
                                prab[:, half, rq:],
                                start=(idx == 0),
                                stop=(idx == nkt - 1),
                            )

                    for idx, kt in enumerate(kt_order):
                        ktsl = slice(kt * P, (kt + 1) * P)
                        r = kt - 4 * qt
                        rq = max(r, 0) * P  # causally-valid q range start
                        pstile = ps_s.tile([P, 2, QTILE], f32, tag="s", name="ps")
                        if "noscores" in probe:
                            nc.vector.memset(pstile[:, :, rq:rq + 1], 0.0)
                        if "noscores" not in probe:
                            for half in range(2):
                                psl = slice(half * 64, (half + 1) * 64)
                                nc.tensor.matmul(
                                    pstile[:, half, rq:],
                                    ksb[psl, hp, ktsl],
                                    qsb[psl, hp, qt * QTILE + rq:(qt + 1) * QTILE],
                                    start=True,
                                    stop=True,
                                    tile_position=(half * 64, 0),
                                )
                        prab = probsp.tile([P, 2, QTILE], bf16, tag="pr", name="pr")
                        if "noexp" not in probe:
                            nc.scalar.activation(
                                prab[:, :, rq:], pstile[:, :, rq:], Exp, scale=0.125
                            )
                        else:
                            nc.vector.memset(prab[:, :, rq:rq + 1], 1.0)
                        if r >= 0:
                            # mask only the diagonal 128-wide band
                            nc.vector.tensor_mul(
                                prab[:, :, rq:rq + P],
                                prab[:, :, rq:rq + P],
                                maskt[:, None, :].to_broadcast([P, 2, P]),
                            )
                        pending.append((idx, prab, rq))
                        pvd = pvdepth
                        if final and hp == 1:
                            # flush PVs eagerly near the end: the last norm +
                            # output projection chain starts sooner
                            pvd = opts.get("pvdepth_final", pvdepth)
                        if len(pending) > min(pvd, nkt - 2):
                            emit_pv(*pending.pop(0))
                        pace(slot_idx)
                        slot_idx += 1
                        for cl in deferq.pop(slot_idx, ()):
                            cl()
                    for args in pending:
                        emit_pv(*args)
                        # a filler between flushed PVs: each PV may wait on
                        # its exp, so give the PE other work in between
                        if emitted[0] < nfill - keep_tail:
                            fillers[emitted[0]]()
                            emitted[0] += 1

                    def norm_chain(hp=hp, po=po):
                        for half in range(2):
                            rc = yp.tile([1, QTILE], f32, tag="rc", name="rc")
                            nc.vector.reciprocal(rc[:], po[half][DK:DK + 1, :])
                            rb = yp.tile([DK, QTILE], f32, tag="rb", name="rb")
                            nc.gpsimd.partition_broadcast(
                                rb[:], rc[:], channels=DK
                            )
                            nc.vector.tensor_mul(
                                onorm[64 * half:64 * half + 64, hp, :],
                                po[half][0:DK, :],
                                rb[:],
                            )
                        if onorm8 is not None:
                            nc.gpsimd.tensor_copy(
                                onorm8[:, hp, :], onorm[:, hp, :]
                            )
                            nc.gpsimd.tensor_sub(
                                rho8[:, hp, :], onorm[:, hp, :], onorm8[:, hp, :]
                            )

                    if hp == 0:
                        # emit a few slots into hp1 so it doesn't block hp1's
                        # diagonal mask multiplies in the DVE queue
                        deferq.setdefault(nkt + defer_at, []).append(norm_chain)
                    else:
                        deferred = norm_chain

                for cls in deferq.values():  # anything not yet released
                    for cl in cls:
                        cl()
                if final:
                    # norm chain ahead of the held-back fillers' DVE copies,
                    # which then give the PE work during its DVE/Pool latency
                    deferred()
                    deferred = None
                while emitted[0] < nfill:
                    fillers[emitted[0]]()
                    emitted[0] += 1
                return (onorm, onorm8, rho8), deferred

            # software pipeline: round t runs attention(t) interleaved with
            # filler matmul chains. The projections of tile t+1 fill round t;
            # output projections are pushed two rounds late (oproj(t) fills
            # round t+2) because the last round is exp-bound on the
            # Activation engine and needs all the spare PE work it can get.
            def body():
                # tile-0 projections, split in 256-token halves so the first
                # chain starts as soon as the first xt half lands; the second
                # warmup burst bridges until the mt1-3 weights and second xt
                # half arrive (hp0's rope deps, mt 0 and 2, come first so
                # round 0's scores unblock early)
                n_warm2 = opts.get("n_warm2", 6)
                qkv_chain(0, 0, 0, 256, cpy_act=True)
                if n_warm2:
                    wps2 = ps_s.tile([P, 256], f32, tag="s", name="warm2")
                    for i in range(n_warm2):
                        nc.tensor.matmul(
                            wps2[:],
                            scratch[:, 0:P],
                            scratch[:],
                            start=(i == 0),
                            stop=(i == n_warm2 - 1),
                        )
                for mt in (2, 1, 3):
                    qkv_chain(0, mt, 0, 256, cpy_act=True)
                for mt in (0, 2, 1, 3):
                    qkv_chain(0, mt, 256, QTILE, cpy_act=True)
                for tt in range(4):
                    v_chain(0, tt, cpy_act=True)
                onorms = {}
                pre = ()
                # which earlier tiles' output projections fill each round
                oproj_sched = opts.get("oproj_sched", {2: (0, 1), 3: (2,)})
                for t in range(NQ):
                    fillers = []
                    if t + 1 < NQ:
                        fillers += phase_b_fillers(t + 1)
                    for qo in oproj_sched.get(t, ()):
                        # the last round's held-back tail chains rotate
                        # through both free psum pools so they aren't
                        # copy-paced through a single 2-slot ring
                        pl = (
                            ((ps_m, "m"), (ps_m, "m"), (ps_s, "s"), (ps_m, "m"),
                             (ps_s, "s"), (ps_m, "m"), (ps_s, "s"), (ps_m, "m"))
                            if t == NQ - 1
                            else ((ps_m, "m"),)
                        )
                        fillers += oproj_fillers(qo, onorms.pop(qo), pools=pl)
                    onorms[t], deferred = run_round(
                        t, fillers, pre=pre,
                        keep_tail=opts.get("keep_tail", 8) if t == NQ - 1 else 2,
                        final=t == NQ - 1,
                    )
                    pre = (deferred,) if deferred is not None else ()
                # final output projection: the kj=0 halves only need hp0's
                # normalized output (ready mid-round), so they run during the
                # hp1 norm chain's DVE/Pool latency; kj=1 + copies follow
                qt = NQ - 1
                onorm = onorms[qt][0]
                qsl = slice(qt * QTILE, (qt + 1) * QTILE)
                ysb = ysbp.tile([P, KO, QTILE], bf16, tag="y", name="ysb")
                pools4 = [(ps_m, "m"), (ps_s, "s")] * 2
                pys = []
                for ot in range(4):
                    pool, ptag = pools4[ot]
                    py = pool.tile([P, QTILE], f32, tag=ptag, name="py")
                    nc.tensor.matmul(
                        py[:], wo[:, 0, ot * P:(ot + 1) * P], onorm[:, 0, :],
                        start=True, stop=False,
                    )
                    pys.append(py)

                def fin_copy(ot):
                    if ot % 2 == 1:
                        nc.scalar.copy(ysb[:, ot, :], pys[ot][:])
                    else:
                        nc.vector.tensor_copy(ysb[:, ot, :], pys[ot][:])

                for ot in range(4):
                    nc.tensor.matmul(
                        pys[ot][:], wo[:, 1, ot * P:(ot + 1) * P],
                        onorm[:, 1, :], start=False, stop=True,
                    )
                    fin_copy(ot)
                    if ot % 2 == 1:  # ship every pair as soon as it's staged
                        nc.sync.dma_start(
                            out_d[:, ot - 1:ot + 1, qsl], ysb[:, ot - 1:ot + 1, :]
                        )
                for ot in range(4, KO):
                    pool, ptag = pools4[ot - 4]
                    py = pool.tile([P, QTILE], f32, tag=ptag, name="py")
                    for kj in range(2):
                        nc.tensor.matmul(
                            py[:], wo[:, kj, ot * P:(ot + 1) * P],
                            onorm[:, kj, :], start=(kj == 0), stop=(kj == 1),
                        )
                    pys.append(py)
                    fin_copy(ot)
                    if opts.get("fin_dma", "pairs") == "pairs":
                        if ot % 2 == 1:
                            nc.sync.dma_start(
                                out_d[:, ot - 1:ot + 1, qsl], ysb[:, ot - 1:ot + 1, :]
                            )
                    else:
                        if ot == 5:
                            nc.sync.dma_start(out_d[:, 4:6, qsl], ysb[:, 4:6, :])
                        elif ot == 6:
                            nc.sync.dma_start(out_d[:, 6:7, qsl], ysb[:, 6:7, :])
                        elif ot == KO - 1:
                            nc.sync.dma_start(out_d[:, 7:8, qsl], ysb[:, 7:8, :])

            if loop:
                with tc.For_i(0, reps, 1):
                    body()
            else:
                for _rep in range(reps):
                    body()
    nc.compile()
    return nc


def _feature_major(rows_x_d, dt=BF):
    """[M, D] (row-major, d = ko*128+ki) -> [P, KO, M] in dtype dt."""
    m = rows_x_d.shape[0]
    return np.ascontiguousarray(
        rows_x_d.T.reshape(KO, P, m).transpose(1, 0, 2)
    ).astype(dt)


def _f8_pair(a):
    """Quantize float32 array to (fp8, fp8 residual)."""
    a8 = a.astype(F8)
    r8 = (a - a8.astype(np.float32)).astype(F8)
    return a8, r8


def _prep_in_maps(x, W_qkv, W_o, token_positions):
    x = np.asarray(x, dtype=np.float32)
    W_qkv = np.asarray(W_qkv, dtype=np.float32)
    W_o = np.asarray(W_o, dtype=np.float32)
    pos = np.asarray(token_positions)

    inv_freq = 1.0 / (
        np.float32(THETA) ** (np.arange(0, DK, 2, dtype=np.float32) / np.float32(DK))
    )
    inv_freq = inv_freq.astype(np.float32)
    freqs = pos.astype(np.float32)[:, :, None] * inv_freq[None, None, :]  # [B,S,32]
    cos = np.cos(freqs).astype(np.float32)
    sin = np.sin(freqs).astype(np.float32)

    jidx = (np.arange(P) % DK) // 2
    sign = np.where(np.arange(P) % 2 == 0, -1.0, 1.0).astype(np.float32)
    # cos/sin tables carry the 1/WSCALE descale of the fp8 QKV psum;
    # combined [P, 2, S] (dim1 = cos, sin) for single-DMA loads
    cs_tab = []
    for b in range(B):
        c = np.ascontiguousarray(cos[b].T[jidx] / WSCALE).astype(BF)
        s = np.ascontiguousarray(sin[b].T[jidx] * sign[:, None] / WSCALE).astype(BF)
        cs_tab.append(np.ascontiguousarray(np.stack([c, s], axis=1)))

    masks = (np.arange(P)[:, None] <= np.arange(P)[None, :]).astype(BF)  # tril^T

    xT2 = []
    for b in range(B):
        fm = np.ascontiguousarray(
            x[b].T.reshape(KO, P, S).transpose(1, 0, 2)
        ).astype(np.float32)
        a8, r8 = _f8_pair(fm)
        # [P, 2, KO, S] -> blocked [P, 2, S//256, KO, 256]
        st = np.stack([a8, r8], axis=1).reshape(P, 2, KO, S // 256, 256)
        xT2.append(np.ascontiguousarray(st.transpose(0, 1, 3, 2, 4)))

    in_maps = []
    for c in range(N_CORES):
        b, hg = divmod(c, 4)
        heads = range(hg * HPC, (hg + 1) * HPC)
        q_rows = np.concatenate([W_qkv[h * DK:(h + 1) * DK] for h in heads])
        k_rows = np.concatenate(
            [W_qkv[D + h * DK:D + (h + 1) * DK] for h in heads]
        )
        v_rows = np.concatenate(
            [W_qkv[2 * D + h * DK:2 * D + (h + 1) * DK] for h in heads]
        )
        wqk_fm = _feature_major(
            np.concatenate([q_rows, k_rows]) * WSCALE, np.float32
        )  # [P, KO, 512]
        # regroup as [P, 4(mt), KO, 128] so each mt slice is one contiguous
        # DMA (the 128-col stationary tiles of the QKV matmul)
        wqk_f = np.ascontiguousarray(
            wqk_fm.reshape(P, KO, 4, P).transpose(0, 2, 1, 3)
        )
        wqk8, wqks8 = _f8_pair(wqk_f)
        wv8, wvs8 = _f8_pair(_feature_major(v_rows * WSCALE, np.float32))
        wo_sub = W_o[:, hg * 256:(hg + 1) * 256]  # [D, 256]
        wo_fm = np.ascontiguousarray(
            wo_sub.T.reshape(2, P, D).transpose(1, 0, 2)
        ).astype(np.float32)  # [P, 2, D]
        # onorm arrives at 8x (ones row = 1/8): bf16 final-tile weights are
        # pre-divided by 8; fp8 weights are pre-multiplied by 8 (net 64x psum)
        wo8, wos8 = _f8_pair(wo_fm * 8.0)
        in_maps.append(
            {
                "xT2": xT2[b],
                "wqk2": np.ascontiguousarray(np.stack([wqk8, wqks8], axis=1)),
                "wv2": np.ascontiguousarray(np.stack([wv8, wvs8], axis=1)),
                "wo": (wo_fm / 8.0).astype(BF),
                "wo2": np.ascontiguousarray(np.stack([wo8, wos8], axis=1)),
                "cossin": cs_tab[b],
                "masks": masks,
            }
        )
    return in_maps


def _get_nc(reps=1, loop=False, probe=(), opts=None):
    key = f"nc{reps}_{loop}_{sorted(probe)}_{sorted((opts or {}).items())}"
    if key not in _CACHE:
        _CACHE[key] = _build_nc(reps, loop, probe, opts)
    return _CACHE[key]


def kernel(x, W_qkv, W_o, token_positions):
    nc = _get_nc()
    in_maps = _prep_in_maps(x, W_qkv, W_o, token_positions)
    res = run_bass_kernel_spmd(nc, in_maps, core_ids=list(range(N_CORES)))
    out = np.zeros((B, S, D), dtype=np.float32)
    for c in range(N_CORES):
        b = c // 4
        # out_t is [P, 8(ot), S] bf16: row d = ot*128 + p of y^T
        yt = np.asarray(res.results[c]["out_t"], dtype=np.float32)
        out[b] += yt.transpose(1, 0, 2).reshape(D, S).T
    return out



# revision 59
# speedup vs baseline: 1.0346x; 1.0008x over previous
"""Causal multi-head self-attention with RoPE on 8 TRN2 NeuronCores.

Sharding: core c handles batch b = c // 4 and heads [4*(c%4), 4*(c%4)+4).
All cores run one SPMD Bass program; per-core behavior comes entirely from
the data (pre-sliced weights, per-batch activations). Each core computes its
4 heads' attention and the partial output projection y^T = W_o_slice^T @ out;
the host sums the 4 partials per batch (the "all-reduce" of the TP split).

Device layout is feature-major throughout: x^T [D, S] feeds QKV as the
moving operand; scores are computed transposed (k on partitions, q free) so
the softmax denominator falls out of a ones-row appended to V in the PV
matmul, and the attention output emerges as out^T [d, q], which is exactly
the moving operand the output projection needs. RoPE is applied on the QKV
PSUM with a pair-swap stream shuffle + host-precomputed cos/sin tables.

The emission is software-pipelined: attention slots for query tile t (whose
per-slot rate is bounded by the Activation engine's exp) are interleaved
with the QKV/V projection chains of tile t+1 and the output projection of
tile t-1, so the Tensor engine always has filler matmuls while exp catches
up. Compute dtype bf16 (fp32 accumulate), f32 in, bf16 partials out (host
accumulates the 4 per-batch partials in f32).

The QKV projection runs in fp8(e4m3) DoubleRow perf mode (0.5 PE cycles per
output row, two 128-deep contraction subtiles per instruction) using a
3-term residual expansion x·W ~= x8·w8 + r8·w8 + x8·s8 with r8 = fp8(x-x8),
s8 = fp8(64W - w8); W is pre-scaled by 64 so its residual clears the e4m3
subnormal floor. All quantization happens host-side; the 1/64 descale is
folded into the cos/sin tables for Q/K and into the V psum-copy multiplier.
"""

import sys

sys.path.insert(0, "/opt/trn_rl_repo")

import numpy as np
import ml_dtypes

import concourse.bass as bass
import concourse.bacc as bacc
import concourse.mybir as mybir
import concourse.tile as tile
from concourse.bass_utils import run_bass_kernel_spmd

B, S, D = 2, 2048, 1024
H, DK = 16, 64
THETA = 10000.0
HPC = 4  # heads per core
P = 128
KO = D // P  # 8 contraction subtiles for the projections
QTILE = 512
NQ = S // QTILE  # 4 query tiles
NKT = S // P  # 16 key-token tiles
N_CORES = 8
BF = ml_dtypes.bfloat16

_PAIRSWAP = [i + 1 if i % 2 == 0 else i - 1 for i in range(32)]

F8 = ml_dtypes.float8_e4m3
WSCALE = 64.0  # host pre-scale on W_qkv so fp8 residuals stay out of subnormals

_CACHE = {}


def _build_nc(reps=1, loop=False, probe=(), opts=None):
    probe = set(probe)
    opts = dict(opts or {})
    vcopy_eng = opts.get("vcopy", "act")
    ycopy_eng = opts.get("ycopy", "dve")
    pvdepth = opts.get("pvdepth", 8)
    probs_bufs = opts.get("probs_bufs", 10)
    rope_bufs = opts.get("rope_bufs", 8)
    yp_bufs = opts.get("yp_bufs", 8)
    on_bufs = opts.get("on_bufs", 3)
    diag_first = opts.get("diag_first", False)
    n_warm = opts.get("n_warm", 12)
    f32 = mybir.dt.float32
    bf16 = mybir.dt.bfloat16
    fp8 = mybir.dt.float8e4
    Exp = mybir.ActivationFunctionType.Exp
    DR = mybir.MatmulPerfMode.DoubleRow

    nc = bacc.Bacc()
    # paired-value/residual tensors are combined on dim1 so each slice is a
    # single DMA (HWDGE generation is a serial 625ns/DMA device); x is blocked
    # by 256-token chunks so every DMA slice is a 2KB contiguous run (descs
    # under 512B pay a 2x DMA latency multiplier)
    NB = S // 256
    xT_d = nc.dram_tensor("xT2", [P, 2, NB, KO, 256], fp8, kind="ExternalInput")
    wqk_d = nc.dram_tensor("wqk2", [P, 2, 4, KO, P], fp8, kind="ExternalInput")
    wv_d = nc.dram_tensor("wv2", [P, 2, KO, HPC * DK], fp8, kind="ExternalInput")
    wo_d = nc.dram_tensor("wo", [P, 2, D], bf16, kind="ExternalInput")
    wo2_d = nc.dram_tensor("wo2", [P, 2, 2, D], fp8, kind="ExternalInput")
    cs_d = nc.dram_tensor("cossin", [P, 2, S], bf16, kind="ExternalInput")
    mask_d = nc.dram_tensor("masks", [P, P], bf16, kind="ExternalInput")
    out_d = nc.dram_tensor("out_t", [P, KO, S], bf16, kind="ExternalOutput")

    with tile.TileContext(nc) as tc:
        with (
            tc.tile_pool(name="const", bufs=1) as cp,
            tc.tile_pool(name="rope", bufs=rope_bufs) as ropep,
            tc.tile_pool(name="probs", bufs=probs_bufs) as probsp,
            tc.tile_pool(name="onp", bufs=on_bufs) as onormp,
            tc.tile_pool(name="on8", bufs=on_bufs) as on8p,
            tc.tile_pool(name="yp", bufs=yp_bufs) as yp,
            tc.tile_pool(name="ysb", bufs=2) as ysbp,
            tc.tile_pool(name="ps_s", bufs=2, space="PSUM") as ps_s,
            tc.tile_pool(name="ps_o", bufs=2, space="PSUM") as ps_o,
            tc.tile_pool(name="ps_m", bufs=2, space="PSUM") as ps_m,
        ):
            xt2 = cp.tile([P, 2, NB, KO, 256], fp8, tag="xT2")
            wqk2 = cp.tile([P, 2, 4, KO, P], fp8, tag="wqk2")
            wv2 = cp.tile([P, 2, KO, HPC * DK], fp8, tag="wv2")
            wo = cp.tile([P, 2, D], bf16, tag="wo")
            wo2 = cp.tile([P, 2, 2, D], fp8, tag="wo2")
            cossin = cp.tile([P, 2, S], bf16, tag="cossin")
            maskt = cp.tile([P, P], bf16, tag="mask")
            vones = cp.tile([P, NKT, HPC, DK + 1], bf16, tag="vones")
            qsb = cp.tile([P, 2, S], bf16, tag="qsb")
            ksb = cp.tile([P, 2, S], bf16, tag="ksb")
            # raw (non-tile) sbuf tensor: dependency-free so the warmup
            # matmuls can start immediately without waiting on a memset
            scratch = nc.alloc_sbuf_tensor("warm_scr", [P, 256], bf16)

            # PE pre-ramp: the p-state model runs matmuls at reduced clock
            # until the engine has been continuously busy ~3us. Warm it up on
            # scratch data while the first input DMAs are in flight; the ramp
            # then carries into the real matmul stream with no idle gap.
            # the ones row is 1/8 so onorm comes out as 8*att, matching the
            # x8 pre-scale of the fp8 output-projection weights.
            # scratch is intentionally never initialized: the warmup matmuls
            # only exist to hold the PE p-state ramp, their psum is never
            # read, and skipping the memset lets the PE start immediately.
            nc.vector.memset(vones[:, :, :, DK:DK + 1], 0.125)
            if n_warm:
                wps = ps_s.tile([P, 256], f32, tag="s", name="warm")
                for i in range(n_warm):
                    nc.tensor.matmul(
                        wps[:],
                        scratch[:, 0:P],
                        scratch[:],
                        start=(i == 0),
                        stop=(i == n_warm - 1),
                    )

            # input DMAs, all on the SP queue. Each DMA costs one 625ns HWDGE
            # slot and its transfer serializes on the single DMA device, so
            # order = need-time: the first QKV chain's operands in fine grain
            # (the chain starts as soon as ko 0 lands and is paced by the
            # rest), then everything else batched coarse.
            # prologue slices in consumption order: chain A (x8*w8) of mt 0
            # starts after DMAs 1-2; the r8/s8 terms and the other mt blocks
            # follow; cos/sin splits so the first rope isn't behind xt blk1
            nc.sync.dma_start(wqk2[:, :, 0], wqk_d[:, :, 0])
            nc.sync.dma_start(xt2[:, 0, 0], xT_d[:, 0, 0])
            nc.sync.dma_start(xt2[:, 1, 0], xT_d[:, 1, 0])
            nc.sync.dma_start(wqk2[:, :, 2], wqk_d[:, :, 2])
            nc.sync.dma_start(cossin[:, :, 0:256], cs_d[:, :, 0:256])
            nc.sync.dma_start(wqk2[:, :, 1], wqk_d[:, :, 1])
            nc.sync.dma_start(wqk2[:, :, 3], wqk_d[:, :, 3])
            nc.sync.dma_start(xt2[:, 0, 1], xT_d[:, 0, 1])
            nc.sync.dma_start(xt2[:, 1, 1], xT_d[:, 1, 1])
            nc.sync.dma_start(cossin[:, :, 256:QTILE], cs_d[:, :, 256:QTILE])
            nc.sync.dma_start(maskt[:], mask_d[:])
            nc.sync.dma_start(wv2[:], wv_d[:])
            for nt in range(1, NQ):
                sl = slice(nt * QTILE, (nt + 1) * QTILE)
                nc.sync.dma_start(xt2[:, :, 2 * nt:2 * nt + 2], xT_d[:, :, 2 * nt:2 * nt + 2])
                nc.sync.dma_start(cossin[:, :, sl], cs_d[:, :, sl])
                if nt == 2:
                    nc.sync.dma_start(wo2[:], wo2_d[:])
                if nt == 3:
                    nc.sync.dma_start(wo[:], wo_d[:])

            def qkv_chain(nt, mt, lo=0, hi=QTILE, cpy_act=False):
                """One 128-col QKV chain + psum copy + rope to qsb/ksb.

                3-term fp8 DoubleRow: x8·w8 + r8·w8 + x8·s8 in 256-col
                chunks (DoubleRow moving free = 2N <= 512)."""
                nsl = slice(nt * QTILE + lo, nt * QTILE + hi)
                w = hi - lo
                pqk = ps_m.tile([P, w], f32, tag="m", name="pqk")
                for c0 in range(0, w, 256):
                    cw = min(256, w - c0)
                    blk = (nt * QTILE + lo + c0) // 256
                    ni = 0
                    for jw, jx in ((0, 0), (0, 1), (1, 0)):
                        for t in range(KO // 2):
                            nc.tensor.matmul(
                                pqk[:, c0:c0 + cw],
                                wqk2[:, jw, mt, 2 * t:2 * t + 2, :],
                                xt2[:, jx, blk, 2 * t:2 * t + 2, 0:cw],
                                start=(ni == 0),
                                stop=(ni == 3 * (KO // 2) - 1),
                                perf_mode=DR,
                            )
                            ni += 1
                cpy = ropep.tile([P, w], bf16, tag="cpy", name="cpy")
                # prologue + early-round chains copy on ACT (exp stream idle
                # or thin there); later rounds keep DVE
                if cpy_act or nt <= opts.get("qcopy_act_nt", 2):
                    nc.scalar.copy(cpy[:], pqk[:])
                else:
                    nc.vector.tensor_copy(cpy[:], pqk[:])
                t0 = ropep.tile([P, w], bf16, tag="t0", name="t0")
                sw = ropep.tile([P, w], bf16, tag="sw", name="sw")
                u0 = ropep.tile([P, w], bf16, tag="u0", name="u0")
                nc.vector.tensor_mul(t0[:], cpy[:], cossin[:, 0, nsl])
                nc.vector.stream_shuffle(sw[:], cpy[:], _PAIRSWAP)
                nc.vector.tensor_mul(u0[:], sw[:], cossin[:, 1, nsl])
                dst = qsb if mt < 2 else ksb
                nc.vector.tensor_add(dst[:, mt % 2, nsl], t0[:], u0[:])

            def v_chain(nt, tt, cpy_act=False):
                """V projection for one 128-token subtile (3-term fp8 DR).
                The psum holds 64·v; the copy descales by 1/64."""
                kt = nt * 4 + tt
                blk, off = kt // 2, (kt % 2) * P
                pv = ps_m.tile([P, HPC, DK], f32, tag="m", name="pv")
                ni = 0
                for jx, jw in ((0, 0), (1, 0), (0, 1)):
                    for t in range(KO // 2):
                        nc.tensor.matmul(
                            pv[:],
                            xt2[:, jx, blk, 2 * t:2 * t + 2, off:off + P],
                            wv2[:, jw, 2 * t:2 * t + 2, :],
                            start=(ni == 0),
                            stop=(ni == 3 * (KO // 2) - 1),
                            perf_mode=DR,
                        )
                        ni += 1
                if vcopy_eng == "dve" and not cpy_act:
                    nc.vector.tensor_scalar_mul(
                        vones[:, kt, :, 0:DK], pv[:], 1.0 / WSCALE
                    )
                else:
                    nc.scalar.mul(vones[:, kt, :, 0:DK], pv[:], 1.0 / WSCALE)

            def phase_b_fillers(nt):
                return [lambda mt=mt: qkv_chain(nt, mt) for mt in range(4)] + [
                    lambda tt=tt: v_chain(nt, tt) for tt in range(4)
                ]

            def oproj_fillers(qt, onorm_pack, pools=((ps_m, "m"),)):
                """Output projection chains for query tile qt (3-term fp8
                DoubleRow; psum lands at 64x so copies descale by 1/64).
                The later tiles run near the kernel tail where the exp
                stream has ended, so their copies alternate onto the idle
                Activation engine and their output DMAs are split to keep
                the last transfer small."""
                onorm, onorm8, rho8 = onorm_pack
                qsl = slice(qt * QTILE, (qt + 1) * QTILE)
                late = qt >= NQ - 2
                last = qt == NQ - 1
                ysb = ysbp.tile([P, KO, QTILE], bf16, tag="y", name="ysb")

                def ochain(ot):
                    pool, ptag = pools[ot % len(pools)]
                    py = pool.tile([P, QTILE], f32, tag=ptag, name="py")
                    otsl = slice(ot * P, (ot + 1) * P)
                    for c0 in (0, 256):
                        for ni, (j, mv) in enumerate(
                            ((0, onorm8), (0, rho8), (1, onorm8))
                        ):
                            nc.tensor.matmul(
                                py[:, c0:c0 + 256],
                                wo2[:, j, 0:2, otsl],
                                mv[:, 0:2, c0:c0 + 256],
                                start=(ni == 0),
                                stop=(ni == 2),
                                perf_mode=DR,
                            )
                    if "noy" in probe:
                        return
                    # Activation copies only where the exp stream has ended
                    # (the held-back tail chains); mid-round copies stay on
                    # DVE so they don't stretch the exp-bound final round
                    use_act = ycopy_eng == "act" or (
                        qt == NQ - 2 and ot >= opts.get("act_ot", 2)
                    )
                    if use_act:
                        nc.scalar.mul(ysb[:, ot, :], py[:], 1.0 / WSCALE)
                    else:
                        nc.vector.tensor_scalar_mul(
                            ysb[:, ot, :], py[:], 1.0 / WSCALE
                        )
                    # split the late tiles' output DMAs so the tail after the
                    # last matmul is a small transfer, not a whole tile; the
                    # qt==NQ-2 tile ships in fine grain because its last
                    # chunks land inside the final tile's drain window
                    if late and not last:
                        if opts.get("late_dma", "fine") == "fine":
                            if ot in (1, 3, 5, 7):
                                nc.sync.dma_start(
                                    out_d[:, ot - 1:ot + 1, qsl],
                                    ysb[:, ot - 1:ot + 1, :],
                                )
                        else:
                            if ot == 3:
                                nc.sync.dma_start(out_d[:, 0:4, qsl], ysb[:, 0:4, :])
                            elif ot == KO - 1:
                                nc.sync.dma_start(out_d[:, 4:8, qsl], ysb[:, 4:8, :])
                    elif last and ot == 5:
                        nc.sync.dma_start(out_d[:, 4:6, qsl], ysb[:, 4:6, :])
                    elif last and ot == 6:
                        nc.sync.dma_start(out_d[:, 6:7, qsl], ysb[:, 6:7, :])
                    elif last and ot == KO - 1:
                        nc.sync.dma_start(out_d[:, 7:8, qsl], ysb[:, 7:8, :])
                    elif not last and ot == KO - 1:
                        nc.sync.dma_start(out_d[:, :, qsl], ysb[:])

                return [lambda ot=ot: ochain(ot) for ot in range(KO)]

            def run_round(qt, fillers, pre=(), defer_at=None, keep_tail=2,
                          final=False):
                if defer_at is None:
                    defer_at = opts.get("defer_at", 3)
                """Attention for query tile qt, interleaved with fillers.

                ``pre`` holds deferred closures (the previous round's softmax
                normalization chains): they are emitted a few slots in, so
                they don't head-of-line-block the DVE queue ahead of this
                round's first diagonal mask multiplies. This round's own norm
                chains are returned as closures for the next round (the last
                round emits them inline). Filler pacing is weighted by each
                slot's PE deficit: a diagonal slot has little matmul work but
                a full exp, so it gets more filler coverage.
                """
                onorm = onormp.tile([P, 2, QTILE], bf16, tag="on", name="onorm")
                if qt < NQ - 1:
                    # fp8 quantized onorm + residual for the fp8 output
                    # projection (produced on the otherwise-idle Pool engine)
                    onorm8 = on8p.tile([P, 2, QTILE], fp8, tag="o8", name="on8")
                    rho8 = on8p.tile([P, 2, QTILE], fp8, tag="r8", name="rho8")
                else:
                    onorm8 = rho8 = None
                nkt = 4 * (qt + 1)
                nfill = len(fillers)
                emitted = [0]
                # deferred closures keyed by the slot index that releases them
                deferq = {defer_at: list(pre)}

                # per-slot PE deficit weight: full-slot ACT time is roughly
                # constant, PE slot work scales with the causal width
                weights = []
                for hp in range(2):
                    for kt in (range(4 * qt, nkt) if diag_first else range(nkt)):
                        r = kt - 4 * qt
                        rq = max(r, 0) * P
                        weights.append(1.0 + 3.0 * rq / QTILE)
                    if diag_first:
                        weights.extend([1.0] * (4 * qt))
                total_w = sum(weights)
                nslots = len(weights)
                cum = 0.0

                front = opts.get("front_final", 1.5) if final else opts.get("front", 1.0)
                # fillers may consume onorm written by the deferred norm
                # closures in ``pre`` — hold them until those are emitted
                fill_gate = defer_at if pre else 0

                def pace(i):
                    nonlocal cum
                    cum += weights[i]
                    if i < fill_gate:
                        return
                    want = min(
                        nfill - keep_tail,
                        int(front * nfill * cum / total_w + 1e-9),
                    )
                    while emitted[0] < want:
                        fillers[emitted[0]]()
                        emitted[0] += 1

                slot_idx = 0
                for hp in range(2):
                    po = [
                        ps_o.tile([P, QTILE], f32, tag="o", name=f"po{i}")
                        for i in range(2)
                    ]
                    if diag_first:
                        kt_order = list(range(4 * qt, nkt)) + list(range(4 * qt))
                    else:
                        kt_order = list(range(nkt))
                    pending = []

                    def emit_pv(idx, prab, rq, po=po, hp=hp, nkt=nkt):
                        if "nopv" in probe:
                            if idx == 0:
                                for half in range(2):
                                    nc.vector.memset(po[half][: DK + 1, 0:1], 1.0)
                            return
                        for half in range(2):
                            h = 2 * hp + half
                            nc.tensor.matmul(
                                po[half][: DK + 1, rq:],
                                vones[:, kt_order[idx], h, :],
                                prab[:, half, rq:],
                                start=(idx == 0),
                                stop=(idx == nkt - 1),
                            )

                    for idx, kt in enumerate(kt_order):
                        ktsl = slice(kt * P, (kt + 1) * P)
                        r = kt - 4 * qt
                        rq = max(r, 0) * P  # causally-valid q range start
                        pstile = ps_s.tile([P, 2, QTILE], f32, tag="s", name="ps")
                        if "noscores" in probe:
                            nc.vector.memset(pstile[:, :, rq:rq + 1], 0.0)
                        if "noscores" not in probe:
                            for half in range(2):
                                psl = slice(half * 64, (half + 1) * 64)
                                nc.tensor.matmul(
                                    pstile[:, half, rq:],
                                    ksb[psl, hp, ktsl],
                                    qsb[psl, hp, qt * QTILE + rq:(qt + 1) * QTILE],
                                    start=True,
                                    stop=True,
                                    tile_position=(half * 64, 0),
                                )
                        prab = probsp.tile([P, 2, QTILE], bf16, tag="pr", name="pr")
                        if "noexp" not in probe:
                            nc.scalar.activation(
                                prab[:, :, rq:], pstile[:, :, rq:], Exp, scale=0.125
                            )
                        else:
                            nc.vector.memset(prab[:, :, rq:rq + 1], 1.0)
                        if r >= 0:
                            # mask only the diagonal 128-wide band
                            nc.vector.tensor_mul(
                                prab[:, :, rq:rq + P],
                                prab[:, :, rq:rq + P],
                                maskt[:, None, :].to_broadcast([P, 2, P]),
                            )
                        pending.append((idx, prab, rq))
                        pvd = pvdepth
                        if final and hp == 1:
                            # flush PVs eagerly near the end: the last norm +
                            # output projection chain starts sooner
                            pvd = opts.get("pvdepth_final", pvdepth)
                        if len(pending) > min(pvd, nkt - 2):
                            emit_pv(*pending.pop(0))
                        pace(slot_idx)
                        slot_idx += 1
                        for cl in deferq.pop(slot_idx, ()):
                            cl()
                    for args in pending:
                        emit_pv(*args)
                        # a filler between flushed PVs: each PV may wait on
                        # its exp, so give the PE other work in between
                        if emitted[0] < nfill - keep_tail:
                            fillers[emitted[0]]()
                            emitted[0] += 1

                    def norm_chain(hp=hp, po=po):
                        for half in range(2):
                            rc = yp.tile([1, QTILE], f32, tag="rc", name="rc")
                            nc.vector.reciprocal(rc[:], po[half][DK:DK + 1, :])
                            rb = yp.tile([DK, QTILE], f32, tag="rb", name="rb")
                            nc.gpsimd.partition_broadcast(
                                rb[:], rc[:], channels=DK
                            )
                            nc.vector.tensor_mul(
                                onorm[64 * half:64 * half + 64, hp, :],
                                po[half][0:DK, :],
                                rb[:],
                            )
                        if onorm8 is not None:
                            nc.gpsimd.tensor_copy(
                                onorm8[:, hp, :], onorm[:, hp, :]
                            )
                            nc.gpsimd.tensor_sub(
                                rho8[:, hp, :], onorm[:, hp, :], onorm8[:, hp, :]
                            )

                    if hp == 0:
                        # emit a few slots into hp1 so it doesn't block hp1's
                        # diagonal mask multiplies in the DVE queue
                        deferq.setdefault(nkt + defer_at, []).append(norm_chain)
                    else:
                        deferred = norm_chain

                for cls in deferq.values():  # anything not yet released
                    for cl in cls:
                        cl()
                if final:
                    # norm chain ahead of the held-back fillers' DVE copies,
                    # which then give the PE work during its DVE/Pool latency
                    deferred()
                    deferred = None
                while emitted[0] < nfill:
                    fillers[emitted[0]]()
                    emitted[0] += 1
                return (onorm, onorm8, rho8), deferred

            # software pipeline: round t runs attention(t) interleaved with
            # filler matmul chains. The projections of tile t+1 fill round t;
            # output projections are pushed two rounds late (oproj(t) fills
            # round t+2) because the last round is exp-bound on the
            # Activation engine and needs all the spare PE work it can get.
            def body():
                # tile-0 projections, split in 256-token halves so the first
                # chain starts as soon as the first xt half lands; the second
                # warmup burst bridges until the mt1-3 weights and second xt
                # half arrive (hp0's rope deps, mt 0 and 2, come first so
                # round 0's scores unblock early)
                n_warm2 = opts.get("n_warm2", 6)
                qkv_chain(0, 0, 0, 256, cpy_act=True)
                if n_warm2:
                    wps2 = ps_s.tile([P, 256], f32, tag="s", name="warm2")
                    for i in range(n_warm2):
                        nc.tensor.matmul(
                            wps2[:],
                            scratch[:, 0:P],
                            scratch[:],
                            start=(i == 0),
                            stop=(i == n_warm2 - 1),
                        )
                for mt in (2, 1, 3):
                    qkv_chain(0, mt, 0, 256, cpy_act=True)
                for mt in (0, 2, 1, 3):
                    qkv_chain(0, mt, 256, QTILE, cpy_act=True)
                for tt in range(4):
                    v_chain(0, tt, cpy_act=True)
                onorms = {}
                pre = ()
                # which earlier tiles' output projections fill each round
                oproj_sched = opts.get("oproj_sched", {2: (0, 1), 3: (2,)})
                for t in range(NQ):
                    fillers = []
                    if t + 1 < NQ:
                        fillers += phase_b_fillers(t + 1)
                    for qo in oproj_sched.get(t, ()):
                        # the last round's held-back tail chains rotate
                        # through both free psum pools so they aren't
                        # copy-paced through a single 2-slot ring
                        pl = (
                            ((ps_m, "m"), (ps_m, "m"), (ps_s, "s"), (ps_m, "m"),
                             (ps_s, "s"), (ps_m, "m"), (ps_s, "s"), (ps_m, "m"))
                            if t == NQ - 1
                            else ((ps_m, "m"),)
                        )
                        fillers += oproj_fillers(qo, onorms.pop(qo), pools=pl)
                    onorms[t], deferred = run_round(
                        t, fillers, pre=pre,
                        keep_tail=opts.get("keep_tail", 8) if t == NQ - 1 else 2,
                        final=t == NQ - 1,
                    )
                    pre = (deferred,) if deferred is not None else ()
                # final output projection: the kj=0 halves only need hp0's
                # normalized output (ready mid-round), so they run during the
                # hp1 norm chain's DVE/Pool latency; kj=1 + copies follow
                qt = NQ - 1
                onorm = onorms[qt][0]
                qsl = slice(qt * QTILE, (qt + 1) * QTILE)
                ysb = ysbp.tile([P, KO, QTILE], bf16, tag="y", name="ysb")
                pools4 = [(ps_m, "m"), (ps_s, "s")] * 2
                pys = []
                for ot in range(4):
                    pool, ptag = pools4[ot]
                    py = pool.tile([P, QTILE], f32, tag=ptag, name="py")
                    nc.tensor.matmul(
                        py[:], wo[:, 0, ot * P:(ot + 1) * P], onorm[:, 0, :],
                        start=True, stop=False,
                    )
                    pys.append(py)

                def fin_copy(ot):
                    if ot % 2 == 1:
                        nc.scalar.copy(ysb[:, ot, :], pys[ot][:])
                    else:
                        nc.vector.tensor_copy(ysb[:, ot, :], pys[ot][:])

                for ot in range(4):
                    nc.tensor.matmul(
                        pys[ot][:], wo[:, 1, ot * P:(ot + 1) * P],
                        onorm[:, 1, :], start=False, stop=True,
                    )
                    fin_copy(ot)
                    if ot % 2 == 1:  # ship every pair as soon as it's staged
                        nc.sync.dma_start(
                            out_d[:, ot - 1:ot + 1, qsl], ysb[:, ot - 1:ot + 1, :]
                        )
                for ot in range(4, KO):
                    pool, ptag = pools4[ot - 4]
                    py = pool.tile([P, QTILE], f32, tag=ptag, name="py")
                    for kj in range(2):
                        nc.tensor.matmul(
                            py[:], wo[:, kj, ot * P:(ot + 1) * P],
                            onorm[:, kj, :], start=(kj == 0), stop=(kj == 1),
                        )
                    pys.append(py)
                    fin_copy(ot)
                    if opts.get("fin_dma", "pairs") == "pairs":
                        if ot % 2 == 1:
                            nc.sync.dma_start(
                                out_d[:, ot - 1:ot + 1, qsl], ysb[:, ot - 1:ot + 1, :]
                            )
                    else:
                        if ot == 5:
                            nc.sync.dma_start(out_d[:, 4:6, qsl], ysb[:, 4:6, :])
                        elif ot == 6:
                            nc.sync.dma_start(out_d[:, 6:7, qsl], ysb[:, 6:7, :])
                        elif ot == KO - 1:
                            nc.sync.dma_start(out_d[:, 7:8, qsl], ysb[:, 7:8, :])

            if loop:
                with tc.For_i(0, reps, 1):
                    body()
            else:
                for _rep in range(reps):
                    body()
    nc.compile()
    return nc


def _feature_major(rows_x_d, dt=BF):
    """[M, D] (row-major, d = ko*128+ki) -> [P, KO, M] in dtype dt."""
    m = rows_x_d.shape[0]
    return np.ascontiguousarray(
        rows_x_d.T.reshape(KO, P, m).transpose(1, 0, 2)
    ).astype(dt)


def _f8_pair(a):
    """Quantize float32 array to (fp8, fp8 residual)."""
    a8 = a.astype(F8)
    r8 = (a - a8.astype(np.float32)).astype(F8)
    return a8, r8


def _prep_in_maps(x, W_qkv, W_o, token_positions):
    x = np.asarray(x, dtype=np.float32)
    W_qkv = np.asarray(W_qkv, dtype=np.float32)
    W_o = np.asarray(W_o, dtype=np.float32)
    pos = np.asarray(token_positions)

    inv_freq = 1.0 / (
        np.float32(THETA) ** (np.arange(0, DK, 2, dtype=np.float32) / np.float32(DK))
    )
    inv_freq = inv_freq.astype(np.float32)
    freqs = pos.astype(np.float32)[:, :, None] * inv_freq[None, None, :]  # [B,S,32]
    cos = np.cos(freqs).astype(np.float32)
    sin = np.sin(freqs).astype(np.float32)

    jidx = (np.arange(P) % DK) // 2
    sign = np.where(np.arange(P) % 2 == 0, -1.0, 1.0).astype(np.float32)
    # cos/sin tables carry the 1/WSCALE descale of the fp8 QKV psum;
    # combined [P, 2, S] (dim1 = cos, sin) for single-DMA loads
    cs_tab = []
    for b in range(B):
        c = np.ascontiguousarray(cos[b].T[jidx] / WSCALE).astype(BF)
        s = np.ascontiguousarray(sin[b].T[jidx] * sign[:, None] / WSCALE).astype(BF)
        cs_tab.append(np.ascontiguousarray(np.stack([c, s], axis=1)))

    masks = (np.arange(P)[:, None] <= np.arange(P)[None, :]).astype(BF)  # tril^T

    xT2 = []
    for b in range(B):
        fm = np.ascontiguousarray(
            x[b].T.reshape(KO, P, S).transpose(1, 0, 2)
        ).astype(np.float32)
        a8, r8 = _f8_pair(fm)
        # [P, 2, KO, S] -> blocked [P, 2, S//256, KO, 256]
        st = np.stack([a8, r8], axis=1).reshape(P, 2, KO, S // 256, 256)
        xT2.append(np.ascontiguousarray(st.transpose(0, 1, 3, 2, 4)))

    in_maps = []
    for c in range(N_CORES):
        b, hg = divmod(c, 4)
        heads = range(hg * HPC, (hg + 1) * HPC)
        q_rows = np.concatenate([W_qkv[h * DK:(h + 1) * DK] for h in heads])
        k_rows = np.concatenate(
            [W_qkv[D + h * DK:D + (h + 1) * DK] for h in heads]
        )
        v_rows = np.concatenate(
            [W_qkv[2 * D + h * DK:2 * D + (h + 1) * DK] for h in heads]
        )
        wqk_fm = _feature_major(
            np.concatenate([q_rows, k_rows]) * WSCALE, np.float32
        )  # [P, KO, 512]
        # regroup as [P, 4(mt), KO, 128] so each mt slice is one contiguous
        # DMA (the 128-col stationary tiles of the QKV matmul)
        wqk_f = np.ascontiguousarray(
            wqk_fm.reshape(P, KO, 4, P).transpose(0, 2, 1, 3)
        )
        wqk8, wqks8 = _f8_pair(wqk_f)
        wv8, wvs8 = _f8_pair(_feature_major(v_rows * WSCALE, np.float32))
        wo_sub = W_o[:, hg * 256:(hg + 1) * 256]  # [D, 256]
        wo_fm = np.ascontiguousarray(
            wo_sub.T.reshape(2, P, D).transpose(1, 0, 2)
        ).astype(np.float32)  # [P, 2, D]
        # onorm arrives at 8x (ones row = 1/8): bf16 final-tile weights are
        # pre-divided by 8; fp8 weights are pre-multiplied by 8 (net 64x psum)
        wo8, wos8 = _f8_pair(wo_fm * 8.0)
        in_maps.append(
            {
                "xT2": xT2[b],
                "wqk2": np.ascontiguousarray(np.stack([wqk8, wqks8], axis=1)),
                "wv2": np.ascontiguousarray(np.stack([wv8, wvs8], axis=1)),
                "wo": (wo_fm / 8.0).astype(BF),
                "wo2": np.ascontiguousarray(np.stack([wo8, wos8], axis=1)),
                "cossin": cs_tab[b],
                "masks": masks,
            }
        )
    return in_maps


def _get_nc(reps=1, loop=False, probe=(), opts=None):
    key = f"nc{reps}_{loop}_{sorted(probe)}_{sorted((opts or {}).items())}"
    if key not in _CACHE:
        _CACHE[key] = _build_nc(reps, loop, probe, opts)
    return _CACHE[key]


def kernel(x, W_qkv, W_o, token_positions):
    nc = _get_nc()
    in_maps = _prep_in_maps(x, W_qkv, W_o, token_positions)
    res = run_bass_kernel_spmd(nc, in_maps, core_ids=list(range(N_CORES)))
    out = np.zeros((B, S, D), dtype=np.float32)
    for c in range(N_CORES):
        b = c // 4
        # out_t is [P, 8(ot), S] bf16: row d = ot*128 + p of y^T
        yt = np.asarray(res.results[c]["out_t"], dtype=np.float32)
        out[b] += yt.transpose(1, 0, 2).reshape(D, S).T
    return out



# revision 62
# speedup vs baseline: 1.0355x; 1.0009x over previous
"""Causal multi-head self-attention with RoPE on 8 TRN2 NeuronCores.

Sharding: core c handles batch b = c // 4 and heads [4*(c%4), 4*(c%4)+4).
All cores run one SPMD Bass program; per-core behavior comes entirely from
the data (pre-sliced weights, per-batch activations). Each core computes its
4 heads' attention and the partial output projection y^T = W_o_slice^T @ out;
the host sums the 4 partials per batch (the "all-reduce" of the TP split).

Device layout is feature-major throughout: x^T [D, S] feeds QKV as the
moving operand; scores are computed transposed (k on partitions, q free) so
the softmax denominator falls out of a ones-row appended to V in the PV
matmul, and the attention output emerges as out^T [d, q], which is exactly
the moving operand the output projection needs. RoPE is applied on the QKV
PSUM with a pair-swap stream shuffle + host-precomputed cos/sin tables.

The emission is software-pipelined: attention slots for query tile t (whose
per-slot rate is bounded by the Activation engine's exp) are interleaved
with the QKV/V projection chains of tile t+1 and the output projection of
tile t-1, so the Tensor engine always has filler matmuls while exp catches
up. Compute dtype bf16 (fp32 accumulate), f32 in, bf16 partials out (host
accumulates the 4 per-batch partials in f32).

The QKV projection runs in fp8(e4m3) DoubleRow perf mode (0.5 PE cycles per
output row, two 128-deep contraction subtiles per instruction) using a
3-term residual expansion x·W ~= x8·w8 + r8·w8 + x8·s8 with r8 = fp8(x-x8),
s8 = fp8(64W - w8); W is pre-scaled by 64 so its residual clears the e4m3
subnormal floor. All quantization happens host-side; the 1/64 descale is
folded into the cos/sin tables for Q/K and into the V psum-copy multiplier.

The output projection for tiles 0..2 uses the same 3-term fp8 DoubleRow
scheme: the vones denominator row is 1/8 so onorm lands at 8x, the fp8
W_o is pre-scaled by 8 (net 64x psum, descaled in the ysb copies), and the
onorm fp8 value+residual pair is produced on the otherwise-idle Pool engine
inside the deferred norm chains. The final tile keeps bf16 (W_o/8) because
its Pool quantize would sit on the kernel's tail critical path.

Scores and PV stay bf16: a plain-fp8 stage costs ~2.7% rms (e4m3) which
blows the 2e-2 gate, and 3-term residuals only pay off when DoubleRow also
halves the instruction count (K >= 256); scores contract over just dk=64.
"""

import sys

sys.path.insert(0, "/opt/trn_rl_repo")

import numpy as np
import ml_dtypes

import concourse.bass as bass
import concourse.bacc as bacc
import concourse.mybir as mybir
import concourse.tile as tile
from concourse.bass_utils import run_bass_kernel_spmd

B, S, D = 2, 2048, 1024
H, DK = 16, 64
THETA = 10000.0
HPC = 4  # heads per core
P = 128
KO = D // P  # 8 contraction subtiles for the projections
QTILE = 512
NQ = S // QTILE  # 4 query tiles
NKT = S // P  # 16 key-token tiles
N_CORES = 8
BF = ml_dtypes.bfloat16

_PAIRSWAP = [i + 1 if i % 2 == 0 else i - 1 for i in range(32)]

F8 = ml_dtypes.float8_e4m3
WSCALE = 64.0  # host pre-scale on W_qkv so fp8 residuals stay out of subnormals

_CACHE = {}


def _build_nc(reps=1, loop=False, probe=(), opts=None):
    probe = set(probe)
    opts = dict(opts or {})
    vcopy_eng = opts.get("vcopy", "act")
    ycopy_eng = opts.get("ycopy", "dve")
    pvdepth = opts.get("pvdepth", 8)
    probs_bufs = opts.get("probs_bufs", 10)
    rope_bufs = opts.get("rope_bufs", 8)
    yp_bufs = opts.get("yp_bufs", 8)
    on_bufs = opts.get("on_bufs", 3)
    diag_first = opts.get("diag_first", False)
    n_warm = opts.get("n_warm", 12)
    f32 = mybir.dt.float32
    bf16 = mybir.dt.bfloat16
    fp8 = mybir.dt.float8e4
    Exp = mybir.ActivationFunctionType.Exp
    DR = mybir.MatmulPerfMode.DoubleRow

    nc = bacc.Bacc()
    # paired-value/residual tensors are combined on dim1 so each slice is a
    # single DMA (HWDGE generation is a serial 625ns/DMA device); x is blocked
    # by 256-token chunks so every DMA slice is a 2KB contiguous run (descs
    # under 512B pay a 2x DMA latency multiplier)
    NB = S // 256
    xT_d = nc.dram_tensor("xT2", [P, 2, NB, KO, 256], fp8, kind="ExternalInput")
    wqk_d = nc.dram_tensor("wqk2", [P, 2, 4, KO, P], fp8, kind="ExternalInput")
    wv_d = nc.dram_tensor("wv2", [P, 2, KO, HPC * DK], fp8, kind="ExternalInput")
    wo_d = nc.dram_tensor("wo", [P, 2, D], bf16, kind="ExternalInput")
    wo2_d = nc.dram_tensor("wo2", [P, 2, 2, D], fp8, kind="ExternalInput")
    cs_d = nc.dram_tensor("cossin", [P, 2, S], bf16, kind="ExternalInput")
    mask_d = nc.dram_tensor("masks", [P, P], bf16, kind="ExternalInput")
    out_d = nc.dram_tensor("out_t", [P, KO, S], bf16, kind="ExternalOutput")

    with tile.TileContext(nc) as tc:
        with (
            tc.tile_pool(name="const", bufs=1) as cp,
            tc.tile_pool(name="rope", bufs=rope_bufs) as ropep,
            tc.tile_pool(name="probs", bufs=probs_bufs) as probsp,
            tc.tile_pool(name="onp", bufs=on_bufs) as onormp,
            tc.tile_pool(name="on8", bufs=on_bufs) as on8p,
            tc.tile_pool(name="yp", bufs=yp_bufs) as yp,
            tc.tile_pool(name="ysb", bufs=2) as ysbp,
            tc.tile_pool(name="ps_s", bufs=2, space="PSUM") as ps_s,
            tc.tile_pool(name="ps_o", bufs=2, space="PSUM") as ps_o,
            tc.tile_pool(name="ps_m", bufs=2, space="PSUM") as ps_m,
        ):
            xt2 = cp.tile([P, 2, NB, KO, 256], fp8, tag="xT2")
            wqk2 = cp.tile([P, 2, 4, KO, P], fp8, tag="wqk2")
            wv2 = cp.tile([P, 2, KO, HPC * DK], fp8, tag="wv2")
            wo = cp.tile([P, 2, D], bf16, tag="wo")
            wo2 = cp.tile([P, 2, 2, D], fp8, tag="wo2")
            cossin = cp.tile([P, 2, S], bf16, tag="cossin")
            maskt = cp.tile([P, P], bf16, tag="mask")
            vones = cp.tile([P, NKT, HPC, DK + 1], bf16, tag="vones")
            qsb = cp.tile([P, 2, S], bf16, tag="qsb")
            ksb = cp.tile([P, 2, S], bf16, tag="ksb")
            # raw (non-tile) sbuf tensor: dependency-free so the warmup
            # matmuls can start immediately without waiting on a memset
            scratch = nc.alloc_sbuf_tensor("warm_scr", [P, 256], bf16)

            # PE pre-ramp: the p-state model runs matmuls at reduced clock
            # until the engine has been continuously busy ~3us. Warm it up on
            # scratch data while the first input DMAs are in flight; the ramp
            # then carries into the real matmul stream with no idle gap.
            # the ones row is 1/8 so onorm comes out as 8*att, matching the
            # x8 pre-scale of the fp8 output-projection weights.
            # scratch is intentionally never initialized: the warmup matmuls
            # only exist to hold the PE p-state ramp, their psum is never
            # read, and skipping the memset lets the PE start immediately.
            nc.vector.memset(vones[:, :, :, DK:DK + 1], 0.125)
            if n_warm:
                wps = ps_s.tile([P, 256], f32, tag="s", name="warm")
                for i in range(n_warm):
                    nc.tensor.matmul(
                        wps[:],
                        scratch[:, 0:P],
                        scratch[:],
                        start=(i == 0),
                        stop=(i == n_warm - 1),
                    )

            # input DMAs, all on the SP queue. Each DMA costs one 625ns HWDGE
            # slot and its transfer serializes on the single DMA device, so
            # order = need-time: the first QKV chain's operands in fine grain
            # (the chain starts as soon as ko 0 lands and is paced by the
            # rest), then everything else batched coarse.
            # prologue slices in consumption order: chain A (x8*w8) of mt 0
            # starts after DMAs 1-2; the r8/s8 terms and the other mt blocks
            # follow; cos/sin splits so the first rope isn't behind xt blk1
            nc.sync.dma_start(wqk2[:, :, 0], wqk_d[:, :, 0])
            nc.sync.dma_start(xt2[:, 0, 0], xT_d[:, 0, 0])
            nc.sync.dma_start(xt2[:, 1, 0], xT_d[:, 1, 0])
            nc.sync.dma_start(wqk2[:, :, 2], wqk_d[:, :, 2])
            nc.sync.dma_start(cossin[:, :, 0:256], cs_d[:, :, 0:256])
            nc.sync.dma_start(wqk2[:, :, 1], wqk_d[:, :, 1])
            nc.sync.dma_start(wqk2[:, :, 3], wqk_d[:, :, 3])
            nc.sync.dma_start(xt2[:, 0, 1], xT_d[:, 0, 1])
            nc.sync.dma_start(xt2[:, 1, 1], xT_d[:, 1, 1])
            nc.sync.dma_start(cossin[:, :, 256:QTILE], cs_d[:, :, 256:QTILE])
            nc.sync.dma_start(maskt[:], mask_d[:])
            nc.sync.dma_start(wv2[:], wv_d[:])
            for nt in range(1, NQ):
                sl = slice(nt * QTILE, (nt + 1) * QTILE)
                nc.sync.dma_start(xt2[:, :, 2 * nt:2 * nt + 2], xT_d[:, :, 2 * nt:2 * nt + 2])
                nc.sync.dma_start(cossin[:, :, sl], cs_d[:, :, sl])
                if nt == 2:
                    nc.sync.dma_start(wo2[:], wo2_d[:])
                if nt == 3:
                    nc.sync.dma_start(wo[:], wo_d[:])

            def qkv_chain(nt, mt, lo=0, hi=QTILE, cpy_act=False):
                """One 128-col QKV chain + psum copy + rope to qsb/ksb.

                3-term fp8 DoubleRow: x8·w8 + r8·w8 + x8·s8 in 256-col
                chunks (DoubleRow moving free = 2N <= 512)."""
                nsl = slice(nt * QTILE + lo, nt * QTILE + hi)
                w = hi - lo
                pqk = ps_m.tile([P, w], f32, tag="m", name="pqk")
                for c0 in range(0, w, 256):
                    cw = min(256, w - c0)
                    blk = (nt * QTILE + lo + c0) // 256
                    ni = 0
                    for jw, jx in ((0, 0), (0, 1), (1, 0)):
                        for t in range(KO // 2):
                            nc.tensor.matmul(
                                pqk[:, c0:c0 + cw],
                                wqk2[:, jw, mt, 2 * t:2 * t + 2, :],
                                xt2[:, jx, blk, 2 * t:2 * t + 2, 0:cw],
                                start=(ni == 0),
                                stop=(ni == 3 * (KO // 2) - 1),
                                perf_mode=DR,
                            )
                            ni += 1
                cpy = ropep.tile([P, w], bf16, tag="cpy", name="cpy")
                # prologue + early-round chains copy on ACT (exp stream idle
                # or thin there); later rounds keep DVE
                if cpy_act or nt <= opts.get("qcopy_act_nt", 2):
                    nc.scalar.copy(cpy[:], pqk[:])
                else:
                    nc.vector.tensor_copy(cpy[:], pqk[:])
                t0 = ropep.tile([P, w], bf16, tag="t0", name="t0")
                sw = ropep.tile([P, w], bf16, tag="sw", name="sw")
                u0 = ropep.tile([P, w], bf16, tag="u0", name="u0")
                nc.vector.tensor_mul(t0[:], cpy[:], cossin[:, 0, nsl])
                nc.vector.stream_shuffle(sw[:], cpy[:], _PAIRSWAP)
                nc.vector.tensor_mul(u0[:], sw[:], cossin[:, 1, nsl])
                dst = qsb if mt < 2 else ksb
                nc.vector.tensor_add(dst[:, mt % 2, nsl], t0[:], u0[:])

            def v_chain(nt, tt, cpy_act=False):
                """V projection for one 128-token subtile (3-term fp8 DR).
                The psum holds 64·v; the copy descales by 1/64."""
                kt = nt * 4 + tt
                blk, off = kt // 2, (kt % 2) * P
                pv = ps_m.tile([P, HPC, DK], f32, tag="m", name="pv")
                ni = 0
                for jx, jw in ((0, 0), (1, 0), (0, 1)):
                    for t in range(KO // 2):
                        nc.tensor.matmul(
                            pv[:],
                            xt2[:, jx, blk, 2 * t:2 * t + 2, off:off + P],
                            wv2[:, jw, 2 * t:2 * t + 2, :],
                            start=(ni == 0),
                            stop=(ni == 3 * (KO // 2) - 1),
                            perf_mode=DR,
                        )
                        ni += 1
                if vcopy_eng == "dve" and not cpy_act:
                    nc.vector.tensor_scalar_mul(
                        vones[:, kt, :, 0:DK], pv[:], 1.0 / WSCALE
                    )
                else:
                    nc.scalar.mul(vones[:, kt, :, 0:DK], pv[:], 1.0 / WSCALE)

            def phase_b_fillers(nt):
                return [lambda mt=mt: qkv_chain(nt, mt) for mt in range(4)] + [
                    lambda tt=tt: v_chain(nt, tt) for tt in range(4)
                ]

            def oproj_fillers(qt, onorm_pack, pools=((ps_m, "m"),)):
                """Output projection chains for query tile qt (3-term fp8
                DoubleRow; psum lands at 64x so copies descale by 1/64).
                The later tiles run near the kernel tail where the exp
                stream has ended, so their copies alternate onto the idle
                Activation engine and their output DMAs are split to keep
                the last transfer small."""
                onorm, onorm8, rho8 = onorm_pack
                qsl = slice(qt * QTILE, (qt + 1) * QTILE)
                late = qt >= NQ - 2
                last = qt == NQ - 1
                ysb = ysbp.tile([P, KO, QTILE], bf16, tag="y", name="ysb")

                def ochain(ot):
                    pool, ptag = pools[ot % len(pools)]
                    py = pool.tile([P, QTILE], f32, tag=ptag, name="py")
                    otsl = slice(ot * P, (ot + 1) * P)
                    for c0 in (0, 256):
                        for ni, (j, mv) in enumerate(
                            ((0, onorm8), (0, rho8), (1, onorm8))
                        ):
                            nc.tensor.matmul(
                                py[:, c0:c0 + 256],
                                wo2[:, j, 0:2, otsl],
                                mv[:, 0:2, c0:c0 + 256],
                                start=(ni == 0),
                                stop=(ni == 2),
                                perf_mode=DR,
                            )
                    if "noy" in probe:
                        return
                    # Activation copies only where the exp stream has ended
                    # (the held-back tail chains); mid-round copies stay on
                    # DVE so they don't stretch the exp-bound final round
                    use_act = ycopy_eng == "act" or (
                        qt == NQ - 2 and ot >= opts.get("act_ot", 2)
                    )
                    if use_act:
                        nc.scalar.mul(ysb[:, ot, :], py[:], 1.0 / WSCALE)
                    else:
                        nc.vector.tensor_scalar_mul(
                            ysb[:, ot, :], py[:], 1.0 / WSCALE
                        )
                    # split the late tiles' output DMAs so the tail after the
                    # last matmul is a small transfer, not a whole tile; the
                    # qt==NQ-2 tile ships in fine grain because its last
                    # chunks land inside the final tile's drain window
                    if late and not last:
                        if opts.get("late_dma", "fine") == "fine":
                            if ot in (1, 3, 5, 7):
                                nc.sync.dma_start(
                                    out_d[:, ot - 1:ot + 1, qsl],
                                    ysb[:, ot - 1:ot + 1, :],
                                )
                        else:
                            if ot == 3:
                                nc.sync.dma_start(out_d[:, 0:4, qsl], ysb[:, 0:4, :])
                            elif ot == KO - 1:
                                nc.sync.dma_start(out_d[:, 4:8, qsl], ysb[:, 4:8, :])
                    elif last and ot == 5:
                        nc.sync.dma_start(out_d[:, 4:6, qsl], ysb[:, 4:6, :])
                    elif last and ot == 6:
                        nc.sync.dma_start(out_d[:, 6:7, qsl], ysb[:, 6:7, :])
                    elif last and ot == KO - 1:
                        nc.sync.dma_start(out_d[:, 7:8, qsl], ysb[:, 7:8, :])
                    elif not last and ot == KO - 1:
                        nc.sync.dma_start(out_d[:, :, qsl], ysb[:])

                return [lambda ot=ot: ochain(ot) for ot in range(KO)]

            def run_round(qt, fillers, pre=(), defer_at=None, keep_tail=2,
                          final=False):
                if defer_at is None:
                    defer_at = opts.get("defer_at", 3)
                """Attention for query tile qt, interleaved with fillers.

                ``pre`` holds deferred closures (the previous round's softmax
                normalization chains): they are emitted a few slots in, so
                they don't head-of-line-block the DVE queue ahead of this
                round's first diagonal mask multiplies. This round's own norm
                chains are returned as closures for the next round (the last
                round emits them inline). Filler pacing is weighted by each
                slot's PE deficit: a diagonal slot has little matmul work but
                a full exp, so it gets more filler coverage.
                """
                onorm = onormp.tile([P, 2, QTILE], bf16, tag="on", name="onorm")
                if qt < NQ - 1:
                    # fp8 quantized onorm + residual for the fp8 output
                    # projection (produced on the otherwise-idle Pool engine)
                    onorm8 = on8p.tile([P, 2, QTILE], fp8, tag="o8", name="on8")
                    rho8 = on8p.tile([P, 2, QTILE], fp8, tag="r8", name="rho8")
                else:
                    onorm8 = rho8 = None
                nkt = 4 * (qt + 1)
                nfill = len(fillers)
                emitted = [0]
                # deferred closures keyed by the slot index that releases them
                deferq = {defer_at: list(pre)}

                # per-slot PE deficit weight: full-slot ACT time is roughly
                # constant, PE slot work scales with the causal width
                weights = []
                for hp in range(2):
                    for kt in (range(4 * qt, nkt) if diag_first else range(nkt)):
                        r = kt - 4 * qt
                        rq = max(r, 0) * P
                        weights.append(1.0 + opts.get("wslope", 3.5) * rq / QTILE)
                    if diag_first:
                        weights.extend([1.0] * (4 * qt))
                total_w = sum(weights)
                nslots = len(weights)
                cum = 0.0

                front = opts.get("front_final", 1.5) if final else opts.get("front", 1.0)
                # fillers may consume onorm written by the deferred norm
                # closures in ``pre`` — hold them until those are emitted
                fill_gate = defer_at if pre else 0

                def pace(i):
                    nonlocal cum
                    cum += weights[i]
                    if i < fill_gate:
                        return
                    want = min(
                        nfill - keep_tail,
                        int(front * nfill * cum / total_w + 1e-9),
                    )
                    while emitted[0] < want:
                        fillers[emitted[0]]()
                        emitted[0] += 1

                slot_idx = 0
                for hp in range(2):
                    po = [
                        ps_o.tile([P, QTILE], f32, tag="o", name=f"po{i}")
                        for i in range(2)
                    ]
                    if diag_first:
                        kt_order = list(range(4 * qt, nkt)) + list(range(4 * qt))
                    else:
                        kt_order = list(range(nkt))
                    pending = []

                    def emit_pv(idx, prab, rq, po=po, hp=hp, nkt=nkt):
                        if "nopv" in probe:
                            if idx == 0:
                                for half in range(2):
                                    nc.vector.memset(po[half][: DK + 1, 0:1], 1.0)
                            return
                        for half in range(2):
                            h = 2 * hp + half
                            nc.tensor.matmul(
                                po[half][: DK + 1, rq:],
                                vones[:, kt_order[idx], h, :],
                                prab[:, half, rq:],
                                start=(idx == 0),
                                stop=(idx == nkt - 1),
                            )

                    for idx, kt in enumerate(kt_order):
                        ktsl = slice(kt * P, (kt + 1) * P)
                        r = kt - 4 * qt
                        rq = max(r, 0) * P  # causally-valid q range start
                        pstile = ps_s.tile([P, 2, QTILE], f32, tag="s", name="ps")
                        if "noscores" in probe:
                            nc.vector.memset(pstile[:, :, rq:rq + 1], 0.0)
                        if "noscores" not in probe:
                            for half in range(2):
                                psl = slice(half * 64, (half + 1) * 64)
                                nc.tensor.matmul(
                                    pstile[:, half, rq:],
                                    ksb[psl, hp, ktsl],
                                    qsb[psl, hp, qt * QTILE + rq:(qt + 1) * QTILE],
                                    start=True,
                                    stop=True,
                                    tile_position=(half * 64, 0),
                                )
                        prab = probsp.tile([P, 2, QTILE], bf16, tag="pr", name="pr")
                        if "noexp" not in probe:
                            nc.scalar.activation(
                                prab[:, :, rq:], pstile[:, :, rq:], Exp, scale=0.125
                            )
                        else:
                            nc.vector.memset(prab[:, :, rq:rq + 1], 1.0)
                        if r >= 0:
                            # mask only the diagonal 128-wide band
                            nc.vector.tensor_mul(
                                prab[:, :, rq:rq + P],
                                prab[:, :, rq:rq + P],
                                maskt[:, None, :].to_broadcast([P, 2, P]),
                            )
                        pending.append((idx, prab, rq))
                        pvd = pvdepth
                        if final and hp == 1:
                            # flush PVs eagerly near the end: the last norm +
                            # output projection chain starts sooner
                            pvd = opts.get("pvdepth_final", pvdepth)
                        if len(pending) > min(pvd, nkt - 2):
                            emit_pv(*pending.pop(0))
                        pace(slot_idx)
                        slot_idx += 1
                        for cl in deferq.pop(slot_idx, ()):
                            cl()
                    for args in pending:
                        emit_pv(*args)
                        # a filler between flushed PVs: each PV may wait on
                        # its exp, so give the PE other work in between
                        if emitted[0] < nfill - keep_tail:
                            fillers[emitted[0]]()
                            emitted[0] += 1

                    def norm_chain(hp=hp, po=po):
                        for half in range(2):
                            rc = yp.tile([1, QTILE], f32, tag="rc", name="rc")
                            nc.vector.reciprocal(rc[:], po[half][DK:DK + 1, :])
                            rb = yp.tile([DK, QTILE], f32, tag="rb", name="rb")
                            nc.gpsimd.partition_broadcast(
                                rb[:], rc[:], channels=DK
                            )
                            nc.vector.tensor_mul(
                                onorm[64 * half:64 * half + 64, hp, :],
                                po[half][0:DK, :],
                                rb[:],
                            )
                        if onorm8 is not None:
                            nc.gpsimd.tensor_copy(
                                onorm8[:, hp, :], onorm[:, hp, :]
                            )
                            nc.gpsimd.tensor_sub(
                                rho8[:, hp, :], onorm[:, hp, :], onorm8[:, hp, :]
                            )

                    if hp == 0:
                        # emit a few slots into hp1 so it doesn't block hp1's
                        # diagonal mask multiplies in the DVE queue
                        deferq.setdefault(nkt + defer_at, []).append(norm_chain)
                    else:
                        deferred = norm_chain

                for cls in deferq.values():  # anything not yet released
                    for cl in cls:
                        cl()
                if final:
                    # norm chain ahead of the held-back fillers' DVE copies,
                    # which then give the PE work during its DVE/Pool latency
                    deferred()
                    deferred = None
                while emitted[0] < nfill:
                    fillers[emitted[0]]()
                    emitted[0] += 1
                return (onorm, onorm8, rho8), deferred

            # software pipeline: round t runs attention(t) interleaved with
            # filler matmul chains. The projections of tile t+1 fill round t;
            # output projections are pushed two rounds late (oproj(t) fills
            # round t+2) because the last round is exp-bound on the
            # Activation engine and needs all the spare PE work it can get.
            def body():
                # tile-0 projections, split in 256-token halves so the first
                # chain starts as soon as the first xt half lands; the second
                # warmup burst bridges until the mt1-3 weights and second xt
                # half arrive (hp0's rope deps, mt 0 and 2, come first so
                # round 0's scores unblock early)
                n_warm2 = opts.get("n_warm2", 6)
                qkv_chain(0, 0, 0, 256, cpy_act=True)
                if n_warm2:
                    wps2 = ps_s.tile([P, 256], f32, tag="s", name="warm2")
                    for i in range(n_warm2):
                        nc.tensor.matmul(
                            wps2[:],
                            scratch[:, 0:P],
                            scratch[:],
                            start=(i == 0),
                            stop=(i == n_warm2 - 1),
                        )
                for mt in (2, 1, 3):
                    qkv_chain(0, mt, 0, 256, cpy_act=True)
                for mt in (0, 2, 1, 3):
                    qkv_chain(0, mt, 256, QTILE, cpy_act=True)
                for tt in range(4):
                    v_chain(0, tt, cpy_act=True)
                onorms = {}
                pre = ()
                # which earlier tiles' output projections fill each round
                oproj_sched = opts.get("oproj_sched", {2: (0, 1), 3: (2,)})
                for t in range(NQ):
                    fillers = []
                    if t + 1 < NQ:
                        fillers += phase_b_fillers(t + 1)
                    for qo in oproj_sched.get(t, ()):
                        # the last round's held-back tail chains rotate
                        # through both free psum pools so they aren't
                        # copy-paced through a single 2-slot ring
                        pl = (
                            ((ps_m, "m"), (ps_m, "m"), (ps_s, "s"), (ps_m, "m"),
                             (ps_s, "s"), (ps_m, "m"), (ps_s, "s"), (ps_m, "m"))
                            if t == NQ - 1
                            else ((ps_m, "m"),)
                        )
                        fillers += oproj_fillers(qo, onorms.pop(qo), pools=pl)
                    onorms[t], deferred = run_round(
                        t, fillers, pre=pre,
                        keep_tail=opts.get("keep_tail", 8) if t == NQ - 1 else 2,
                        final=t == NQ - 1,
                    )
                    pre = (deferred,) if deferred is not None else ()
                # final output projection: the kj=0 halves only need hp0's
                # normalized output (ready mid-round), so they run during the
                # hp1 norm chain's DVE/Pool latency; kj=1 + copies follow
                qt = NQ - 1
                onorm = onorms[qt][0]
                qsl = slice(qt * QTILE, (qt + 1) * QTILE)
                ysb = ysbp.tile([P, KO, QTILE], bf16, tag="y", name="ysb")
                pools4 = [(ps_m, "m"), (ps_s, "s")] * 2
                pys = []
                for ot in range(4):
                    pool, ptag = pools4[ot]
                    py = pool.tile([P, QTILE], f32, tag=ptag, name="py")
                    nc.tensor.matmul(
                        py[:], wo[:, 0, ot * P:(ot + 1) * P], onorm[:, 0, :],
                        start=True, stop=False,
                    )
                    pys.append(py)

                def fin_copy(ot):
                    if ot % 2 == 1:
                        nc.scalar.copy(ysb[:, ot, :], pys[ot][:])
                    else:
                        nc.vector.tensor_copy(ysb[:, ot, :], pys[ot][:])

                for ot in range(4):
                    nc.tensor.matmul(
                        pys[ot][:], wo[:, 1, ot * P:(ot + 1) * P],
                        onorm[:, 1, :], start=False, stop=True,
                    )
                    fin_copy(ot)
                    if ot % 2 == 1:  # ship every pair as soon as it's staged
                        nc.sync.dma_start(
                            out_d[:, ot - 1:ot + 1, qsl], ysb[:, ot - 1:ot + 1, :]
                        )
                for ot in range(4, KO):
                    pool, ptag = pools4[ot - 4]
                    py = pool.tile([P, QTILE], f32, tag=ptag, name="py")
                    for kj in range(2):
                        nc.tensor.matmul(
                            py[:], wo[:, kj, ot * P:(ot + 1) * P],
                            onorm[:, kj, :], start=(kj == 0), stop=(kj == 1),
                        )
                    pys.append(py)
                    fin_copy(ot)
                    if opts.get("fin_dma", "pairs") == "pairs":
                        if ot % 2 == 1:
                            nc.sync.dma_start(
                                out_d[:, ot - 1:ot + 1, qsl], ysb[:, ot - 1:ot + 1, :]
                            )
                    else:
                        if ot == 5:
                            nc.sync.dma_start(out_d[:, 4:6, qsl], ysb[:, 4:6, :])
                        elif ot == 6:
                            nc.sync.dma_start(out_d[:, 6:7, qsl], ysb[:, 6:7, :])
                        elif ot == KO - 1:
                            nc.sync.dma_start(out_d[:, 7:8, qsl], ysb[:, 7:8, :])

            if loop:
                with tc.For_i(0, reps, 1):
                    body()
            else:
                for _rep in range(reps):
                    body()
    nc.compile()
    return nc


def _feature_major(rows_x_d, dt=BF):
    """[M, D] (row-major, d = ko*128+ki) -> [P, KO, M] in dtype dt."""
    m = rows_x_d.shape[0]
    return np.ascontiguousarray(
        rows_x_d.T.reshape(KO, P, m).transpose(1, 0, 2)
    ).astype(dt)


def _f8_pair(a):
    """Quantize float32 array to (fp8, fp8 residual)."""
    a8 = a.astype(F8)
    r8 = (a - a8.astype(np.float32)).astype(F8)
    return a8, r8


def _prep_in_maps(x, W_qkv, W_o, token_positions):
    x = np.asarray(x, dtype=np.float32)
    W_qkv = np.asarray(W_qkv, dtype=np.float32)
    W_o = np.asarray(W_o, dtype=np.float32)
    pos = np.asarray(token_positions)

    inv_freq = 1.0 / (
        np.float32(THETA) ** (np.arange(0, DK, 2, dtype=np.float32) / np.float32(DK))
    )
    inv_freq = inv_freq.astype(np.float32)
    freqs = pos.astype(np.float32)[:, :, None] * inv_freq[None, None, :]  # [B,S,32]
    cos = np.cos(freqs).astype(np.float32)
    sin = np.sin(freqs).astype(np.float32)

    jidx = (np.arange(P) % DK) // 2
    sign = np.where(np.arange(P) % 2 == 0, -1.0, 1.0).astype(np.float32)
    # cos/sin tables carry the 1/WSCALE descale of the fp8 QKV psum;
    # combined [P, 2, S] (dim1 = cos, sin) for single-DMA loads
    cs_tab = []
    for b in range(B):
        c = np.ascontiguousarray(cos[b].T[jidx] / WSCALE).astype(BF)
        s = np.ascontiguousarray(sin[b].T[jidx] * sign[:, None] / WSCALE).astype(BF)
        cs_tab.append(np.ascontiguousarray(np.stack([c, s], axis=1)))

    masks = (np.arange(P)[:, None] <= np.arange(P)[None, :]).astype(BF)  # tril^T

    xT2 = []
    for b in range(B):
        fm = np.ascontiguousarray(
            x[b].T.reshape(KO, P, S).transpose(1, 0, 2)
        ).astype(np.float32)
        a8, r8 = _f8_pair(fm)
        # [P, 2, KO, S] -> blocked [P, 2, S//256, KO, 256]
        st = np.stack([a8, r8], axis=1).reshape(P, 2, KO, S // 256, 256)
        xT2.append(np.ascontiguousarray(st.transpose(0, 1, 3, 2, 4)))

    in_maps = []
    for c in range(N_CORES):
        b, hg = divmod(c, 4)
        heads = range(hg * HPC, (hg + 1) * HPC)
        q_rows = np.concatenate([W_qkv[h * DK:(h + 1) * DK] for h in heads])
        k_rows = np.concatenate(
            [W_qkv[D + h * DK:D + (h + 1) * DK] for h in heads]
        )
        v_rows = np.concatenate(
            [W_qkv[2 * D + h * DK:2 * D + (h + 1) * DK] for h in heads]
        )
        wqk_fm = _feature_major(
            np.concatenate([q_rows, k_rows]) * WSCALE, np.float32
        )  # [P, KO, 512]
        # regroup as [P, 4(mt), KO, 128] so each mt slice is one contiguous
        # DMA (the 128-col stationary tiles of the QKV matmul)
        wqk_f = np.ascontiguousarray(
            wqk_fm.reshape(P, KO, 4, P).transpose(0, 2, 1, 3)
        )
        wqk8, wqks8 = _f8_pair(wqk_f)
        wv8, wvs8 = _f8_pair(_feature_major(v_rows * WSCALE, np.float32))
        wo_sub = W_o[:, hg * 256:(hg + 1) * 256]  # [D, 256]
        wo_fm = np.ascontiguousarray(
            wo_sub.T.reshape(2, P, D).transpose(1, 0, 2)
        ).astype(np.float32)  # [P, 2, D]
        # onorm arrives at 8x (ones row = 1/8): bf16 final-tile weights are
        # pre-divided by 8; fp8 weights are pre-multiplied by 8 (net 64x psum)
        wo8, wos8 = _f8_pair(wo_fm * 8.0)
        in_maps.append(
            {
                "xT2": xT2[b],
                "wqk2": np.ascontiguousarray(np.stack([wqk8, wqks8], axis=1)),
                "wv2": np.ascontiguousarray(np.stack([wv8, wvs8], axis=1)),
                "wo": (wo_fm / 8.0).astype(BF),
                "wo2": np.ascontiguousarray(np.stack([wo8, wos8], axis=1)),
                "cossin": cs_tab[b],
                "masks": masks,
            }
        )
    return in_maps


def _get_nc(reps=1, loop=False, probe=(), opts=None):
    key = f"nc{reps}_{loop}_{sorted(probe)}_{sorted((opts or {}).items())}"
    if key not in _CACHE:
        _CACHE[key] = _build_nc(reps, loop, probe, opts)
    return _CACHE[key]


def kernel(x, W_qkv, W_o, token_positions):
    nc = _get_nc()
    in_maps = _prep_in_maps(x, W_qkv, W_o, token_positions)
    res = run_bass_kernel_spmd(nc, in_maps, core_ids=list(range(N_CORES)))
    out = np.zeros((B, S, D), dtype=np.float32)
    for c in range(N_CORES):
        b = c // 4
        # out_t is [P, 8(ot), S] bf16: row d = ot*128 + p of y^T
        yt = np.asarray(res.results[c]["out_t"], dtype=np.float32)
        out[b] += yt.transpose(1, 0, 2).reshape(D, S).T
    return out

